# revision 1
# baseline (speedup 1.0000x reference)
"""Trainium2 Bass kernel for a 2-layer GCN (GCNConv -> relu -> GCNConv -> relu -> Linear).

Math: with s = deg^-1/2 (deg over dst incl. self-loops), per-edge norm = s[src]*s[dst]:
  h1 = relu( (A_norm @ x) @ W1 + b1 )     (aggregate 4-wide first - linearity)
  h2 = relu( (A_norm @ h1) @ W2 + b2 )
  out = h2 @ Wf + bf

Device strategy (8 cores, nodes sharded by dst, SPMD one program):
  - edges sorted by (supertile, src_chunk, dst_tile, dst), padded per
    (dst_tile, src_chunk) group to multiples of 128; identical schedule on all
    cores (group sizes = max over cores).
  - per 128-edge chunk a selection matrix M[e, d] = (dst_local_e == d) * w_e is
    built from iota/is_equal (w = s_src*s_dst for layer 1 -- folding the
    per-edge source scale into M -- and s_dst for layer 2, whose gathered
    table rows are pre-scaled by s_src). Builds are split DVE/Pool/Act.
  - aggregation via PE with M as the STATIONARY operand (cost ~ F, not 128):
    acc[128 dst, F] += M[128 e, 128 dst].T @ feat[128 e, F]; feat comes from
    the host-gathered x stream (layer 1, 4-wide) or dma_gather of the
    AllGather'ed h1' table (layer 2: raw InstDMAGatherAnt with elem_size=64,
    stride 256B -- 128B payload per descriptor, half the wrapper's minimum --
    int16 idx within 25k-row src chunk).
  - node-major dense stages: agg -> PE transpose -> W matmul -> bias/relu.
  - AllGather in 2 halves, emitted inside the layer-1 loop right after the
    supertile completing each half so the Pool sequencer reaches them without
    draining all of layer 1; layer-2 src chunks 0,1 run against half 0 while
    half 1 is still collecting.
"""
import numpy as np
from contextlib import ExitStack
from dataclasses import dataclass


@dataclass(frozen=True)
class Cfg:
    n_nodes: int = 100000
    n_cores: int = 8
    f_in: int = 4
    f_hid: int = 64
    f_out: int = 2
    src_chunks: int = 4
    st_tiles: int = 4

    @property
    def shard(self):
        return self.n_nodes // self.n_cores

    @property
    def n_tiles(self):
        return (self.shard + 127) // 128

    @property
    def last_rows(self):
        return self.shard - (self.n_tiles - 1) * 128

    @property
    def src_chunk(self):
        return self.n_nodes // self.src_chunks

    @property
    def n_st(self):
        return (self.n_tiles + self.st_tiles - 1) // self.st_tiles

    def tiles_of_st(self, st):
        return list(range(st * self.st_tiles, min((st + 1) * self.st_tiles, self.n_tiles)))


CFG = Cfg()
P = 128
FP = 128  # fp16 table row padded to 256B (dma_gather min elem)

# M-build engine split: counter mod 29 -> <20 DVE, <26 Pool, else Act(2-instr)
DVE_CUT, POOL_CUT, SPLIT_MOD = 20, 26, 29


def preprocess(cfg, x, edge_index, W1, b1, W2, b2, Wf, bf):
    """Host-side sharding: sort/group/pad edges, build per-core device arrays."""
    F_IN = cfg.f_in
    src = np.asarray(edge_index[0], dtype=np.int64)
    dst = np.asarray(edge_index[1], dtype=np.int64)
    deg = (np.bincount(dst, minlength=cfg.n_nodes) + 1).astype(np.float64)
    s = (1.0 / np.sqrt(deg)).astype(np.float32)

    core_id = dst // cfg.shard
    x = np.asarray(x, dtype=np.float32)
    # Gather table = 2 AllGather halves; half h holds concat over cores of local
    # rows [h*hs,(h+1)*hs), split into 2 idx chunks each.
    hs = cfg.shard // 2
    cj = hs * cfg.n_cores // 2         # rows per idx chunk
    assert cj <= 32768 and hs * 2 == cfg.shard and cfg.src_chunks == 4

    # group sequence (st, c, t) shared by all cores
    seq = []
    for st in range(cfg.n_st):
        for c in range(cfg.src_chunks):
            for t in cfg.tiles_of_st(st):
                seq.append((t, c))
    n_grp = len(seq)

    grp_base = np.zeros(cfg.n_st, dtype=np.int64)
    acc = 0
    for st in range(cfg.n_st):
        grp_base[st] = acc
        acc += cfg.src_chunks * len(cfg.tiles_of_st(st))

    per_core = []
    counts = np.zeros((cfg.n_cores, n_grp), dtype=np.int64)
    for cpu in range(cfg.n_cores):
        m = core_id == cpu
        sc, dc = src[m], dst[m]
        dl = dc - cpu * cfg.shard
        tl = dl // P
        lr = sc % cfg.shard
        score = sc // cfg.shard
        h = lr // hs
        trow = score * hs + (lr - h * hs)
        ch = h * 2 + trow // cj
        stl = tl // cfg.st_tiles
        order = np.lexsort((dl, tl, ch, stl))
        sc, dc, dl, tl, ch, stl = (a[order] for a in (sc, dc, dl, tl, ch, stl))
        t_in_st = tl % cfg.st_tiles
        tiles_in_st = np.minimum(cfg.st_tiles, cfg.n_tiles - stl * cfg.st_tiles)
        gseq = grp_base[stl] + ch * tiles_in_st + t_in_st
        counts[cpu] = np.bincount(gseq, minlength=n_grp)
        per_core.append((sc, dc, dl, tl, gseq))

    C = (np.ceil(counts.max(axis=0) / P)).astype(np.int64)
    C = np.maximum(C, 1)
    col_off = np.zeros(n_grp + 1, dtype=np.int64)
    np.cumsum(C, out=col_off[1:])
    NCOLS = int(col_off[-1])
    EPAD = NCOLS * P
    gmax = counts.max(axis=0)  # exact per-group idx count (max over cores)

    # schedule metadata
    g_i = 0
    st_meta = []
    tile_chunks = [[] for _ in range(cfg.n_tiles)]
    for st in range(cfg.n_st):
        row = []
        for c in range(cfg.src_chunks):
            tiles = cfg.tiles_of_st(st)
            colstart = int(col_off[g_i])
            G = int(sum(C[g_i + k] for k in range(len(tiles))))
            for k, t in enumerate(tiles):
                g = g_i + k
                base = int(col_off[g])
                for j in range(int(C[g])):
                    tile_chunks[t].append((base + j, st, c, base + j - colstart))
            g_i += len(tiles)
            row.append((colstart, G, G * P))
        st_meta.append(row)

    dev = []
    for cpu in range(cfg.n_cores):
        sc, dc, dl, tl, gseq = per_core[cpu]
        n = len(sc)
        starts = np.zeros(n_grp, dtype=np.int64)
        starts[1:] = np.cumsum(counts[cpu])[:-1]
        rank = np.arange(n) - starts[gseq]
        pos = col_off[gseq] * P + rank

        lr2 = sc % cfg.shard
        score2 = sc // cfg.shard
        h2 = lr2 // hs
        trow2 = score2 * hs + (lr2 - h2 * hs)
        idxl = np.zeros(EPAD, dtype=np.int16)
        idxl[pos] = (trow2 % cj).astype(np.int16)
        dstv = np.full(EPAD, -1.0, dtype=np.float32)
        dstv[pos] = (dl - tl * P).astype(np.float32)
        s1 = np.zeros(EPAD, dtype=np.float32)   # s_dst * s_src  (layer 1)
        s1[pos] = s[dc] * s[sc]
        s2 = np.zeros(EPAD, dtype=np.float32)   # s_dst          (layer 2)
        s2[pos] = s[dc]
        xsrc = np.zeros((EPAD, F_IN), dtype=np.float16)
        xsrc[pos] = x[sc].astype(np.float16)

        xs = np.ascontiguousarray(xsrc.reshape(NCOLS, P, F_IN).transpose(1, 0, 2))
        ds = np.empty((P, NCOLS, 5), dtype=np.float32)
        ds[:, :, 0] = dstv.reshape(NCOLS, P).T
        ds[:, :, 1] = s1.reshape(NCOLS, P).T
        ds[:, :, 2] = s2.reshape(NCOLS, P).T
        ds[:, :, 3] = -ds[:, :, 1]
        ds[:, :, 4] = -ds[:, :, 2]
        idx_w = np.tile(idxl.reshape(NCOLS * 8, 16).T, (8, 1))

        s_core = np.zeros(cfg.n_tiles * P, dtype=np.float32)
        s_core[:cfg.shard] = s[cpu * cfg.shard:(cpu + 1) * cfg.shard]
        s_nm = s_core.reshape(cfg.n_tiles, P).T.copy()
        s2_nm = s_nm * s_nm

        x_core = np.zeros((cfg.n_tiles * P, F_IN), dtype=np.float16)
        x_core[:cfg.shard] = x[cpu * cfg.shard:(cpu + 1) * cfg.shard].astype(np.float16)
        xown = x_core.reshape(cfg.n_tiles, P, F_IN).transpose(1, 0, 2).copy()

        dev.append(dict(xs=xs, ds=ds, idx=np.ascontiguousarray(idx_w),
                        s_nm=s_nm, s2_nm=s2_nm, xown=xown))

    wb = dict(
        W1=np.asarray(W1, np.float32), W2=np.asarray(W2, np.float32),
        Wf=np.asarray(Wf, np.float32),
        B1=np.broadcast_to(np.asarray(b1, np.float32).reshape(1, cfg.f_hid),
                           (P, cfg.f_hid)).copy(),
        b2=np.asarray(b2, np.float32).reshape(cfg.f_hid, 1),
        bf=np.asarray(bf, np.float32).reshape(cfg.f_out, 1),
    )
    sched = dict(NCOLS=NCOLS, st_meta=st_meta, tile_chunks=tile_chunks)
    return dev, wb, sched


def build(cfg, sched):
    import concourse.bass as bass
    import concourse.mybir as mybir
    import concourse.tile as tile
    from concourse import bacc

    dt = mybir.dt
    F_IN, F_HID, F_OUT = cfg.f_in, cfg.f_hid, cfg.f_out
    ncols = sched["NCOLS"]
    st_meta = sched["st_meta"]
    tile_chunks = sched["tile_chunks"]

    nc = bacc.Bacc("TRN2", target_bir_lowering=False, num_devices=cfg.n_cores)
    xs_in = nc.declare_dram_parameter("xs", [P, ncols, F_IN], dt.float16, isOutput=False)
    ds_in = nc.declare_dram_parameter("ds", [P, ncols, 5], dt.float32, isOutput=False)
    idx_in = nc.declare_dram_parameter("idx", [P, ncols * 8], dt.int16, isOutput=False)
    snm_in = nc.declare_dram_parameter("s_nm", [P, cfg.n_tiles], dt.float32, isOutput=False)
    s2nm_in = nc.declare_dram_parameter("s2_nm", [P, cfg.n_tiles], dt.float32, isOutput=False)
    xown_in = nc.declare_dram_parameter("xown", [P, cfg.n_tiles, F_IN], dt.float16, isOutput=False)
    W1_in = nc.declare_dram_parameter("W1", [F_IN, F_HID], dt.float32, isOutput=False)
    W2_in = nc.declare_dram_parameter("W2", [F_HID, F_HID], dt.float32, isOutput=False)
    Wf_in = nc.declare_dram_parameter("Wf", [F_HID, F_OUT], dt.float32, isOutput=False)
    B1_in = nc.declare_dram_parameter("B1", [P, F_HID], dt.float32, isOutput=False)
    b2_in = nc.declare_dram_parameter("b2", [F_HID, 1], dt.float32, isOutput=False)
    bf_in = nc.declare_dram_parameter("bf", [F_OUT, 1], dt.float32, isOutput=False)
    out_ext = nc.declare_dram_parameter("out_fm", [F_OUT, cfg.shard], dt.float32, isOutput=True)

    mb_ci = [0]  # global M-build counter for the engine split

    def thin_gather(out_ap, in_ap, idxs_ap, num_idxs):
        """dma_gather fetching the first 128B of each 256B-strided table row.

        elem_size=64 elements (128B payload) with a 256B row stride halves the
        per-descriptor DMA cost vs the 256B wrapper minimum; verified on HW.
        """
        eng = nc.gpsimd
        _in_ap = eng.lower_ap_dma(in_ap, for_custom_bir_dma=True)
        _idxs_ap = eng.lower_ap(idxs_ap)
        _out_ap = eng.lower_ap(out_ap)
        return eng.add_instruction(
            mybir.InstDMAGatherAnt(
                name=eng.bass.get_next_instruction_name(),
                ins=[*_in_ap, _idxs_ap, eng.lower_val_access(eng.to_reg(num_idxs))],
                outs=[_out_ap],
                transpose=False,
                num_idxs=num_idxs,
                elem_size=F_HID,
                stride_bytes_256=1,
                gen_mode=0,
                single_packet=False,
                queue_num=0,
                sbuf_tokens_per_rank=0,
                sbuf_free_dim_per_rank=0,
                sbuf_free_dim_pad_per_rank=0,
                sbuf_byte_offset=0,
            )
        )

    with tile.TileContext(nc, num_cores=cfg.n_cores) as tc, ExitStack() as ctx:
        dram = ctx.enter_context(tc.tile_pool(name="dram", bufs=1, space="DRAM"))
        const = ctx.enter_context(tc.tile_pool(name="const", bufs=1))
        mpool = ctx.enter_context(tc.tile_pool(name="mpool", bufs=12))
        evac = ctx.enter_context(tc.tile_pool(name="evac", bufs=6))

        h1loc = dram.tile([cfg.shard, FP], dt.float16)
        hrows = cfg.n_nodes // 2
        h1tab0 = dram.tile([hrows, FP], dt.float16, name="h1tab0")
        h1tab1 = dram.tile([hrows, FP], dt.float16, name="h1tab1")

        iota_i = const.tile([P, P], dt.int16)
        nc.gpsimd.iota(iota_i[:], pattern=[[1, P]], base=0, channel_multiplier=0)
        iota16 = const.tile([P, P], dt.float16)
        nc.vector.tensor_copy(iota16[:], iota_i[:])
        iotapP = const.tile([P, 1], dt.int16)
        nc.gpsimd.iota(iotapP[:], pattern=[[0, 1]], base=0, channel_multiplier=1)
        iotapPf = const.tile([P, 1], dt.float32)
        nc.vector.tensor_copy(iotapPf[:], iotapP[:])
        # fp16 128x128 identity for PE transposes
        identP = const.tile([P, P], dt.float16)
        nc.vector.tensor_scalar(out=identP[:], in0=iota16[:], scalar1=iotapPf[:, 0:1],
                                scalar2=None, op0=mybir.AluOpType.is_equal)

        W1s = const.tile([F_IN, F_HID], dt.float32)
        W2s = const.tile([F_HID, F_HID], dt.float32)
        Wfs = const.tile([F_HID, F_OUT], dt.float32)
        B1s = const.tile([P, F_HID], dt.float32)
        b2s = const.tile([F_HID, 1], dt.float32)
        bfs = const.tile([F_OUT, 1], dt.float32)
        snm = const.tile([P, cfg.n_tiles], dt.float32)
        s2nm = const.tile([P, cfg.n_tiles], dt.float32)
        nc.sync.dma_start(W1s[:], W1_in[:])
        nc.sync.dma_start(W2s[:], W2_in[:])
        nc.sync.dma_start(Wfs[:], Wf_in[:])
        nc.sync.dma_start(B1s[:], B1_in[:])
        nc.sync.dma_start(b2s[:], b2_in[:])
        nc.sync.dma_start(bfs[:], bf_in[:])
        nc.sync.dma_start(snm[:], snm_in[:])
        nc.sync.dma_start(s2nm[:], s2nm_in[:])

        ds_all = const.tile([P, ncols, 5], dt.float32)
        nc.sync.dma_start(ds_all[:], ds_in[:])
        W1s16 = const.tile([F_IN, F_HID], dt.float16)
        nc.scalar.activation(W1s16[:], W1s[:], mybir.ActivationFunctionType.Copy)
        W2s16 = const.tile([F_HID, F_HID], dt.float16)
        nc.scalar.activation(W2s16[:], W2s[:], mybir.ActivationFunctionType.Copy)
        Wfs16 = const.tile([F_HID, F_OUT], dt.float16)
        nc.scalar.activation(Wfs16[:], Wfs[:], mybir.ActivationFunctionType.Copy)

        xown16 = const.tile([P, cfg.n_tiles, F_IN], dt.float16)
        nc.sync.dma_start(xown16[:], xown_in[:])
        h1keep = const.tile([P, cfg.n_tiles, F_HID], dt.float16)

        def build_M(scol, wcol, nwcol, no_pool=False):
            """Selection matrix [128e, 128d] = (iota==dstv) * w, split across engines."""
            ci = mb_ci[0]
            mb_ci[0] += 1
            M16 = mpool.tile([P, P], dt.float16, tag="M")
            if no_pool:
                if ci % 8 < 7:
                    nc.vector.tensor_scalar(
                        out=M16[:], in0=iota16[:], scalar1=scol, scalar2=wcol,
                        op0=mybir.AluOpType.is_equal, op1=mybir.AluOpType.mult)
                else:
                    A16 = mpool.tile([P, P], dt.float16, tag="MA")
                    nc.scalar.activation(A16[:], iota16[:], mybir.ActivationFunctionType.Abs,
                                         bias=scol, scale=-1.0)
                    nc.scalar.activation(M16[:], A16[:], mybir.ActivationFunctionType.Relu,
                                         bias=wcol, scale=nwcol)
                return M16
            r = ci % SPLIT_MOD
            if r < DVE_CUT:
                nc.vector.tensor_scalar(
                    out=M16[:], in0=iota16[:], scalar1=scol, scalar2=wcol,
                    op0=mybir.AluOpType.is_equal, op1=mybir.AluOpType.mult)
            elif r < POOL_CUT:
                nc.gpsimd.tensor_scalar(
                    out=M16[:], in0=iota16[:], scalar1=scol, scalar2=wcol,
                    op0=mybir.AluOpType.is_equal, op1=mybir.AluOpType.mult)
            else:
                A16 = mpool.tile([P, P], dt.float16, tag="MA")
                nc.scalar.activation(A16[:], iota16[:], mybir.ActivationFunctionType.Abs,
                                     bias=scol, scale=-1.0)
                nc.scalar.activation(M16[:], A16[:], mybir.ActivationFunctionType.Relu,
                                     bias=wcol, scale=nwcol)
            return M16

        def build_Mdiag(wcol):
            M16 = mpool.tile([P, P], dt.float16, tag="M")
            nc.vector.tensor_scalar(
                out=M16[:], in0=iota16[:], scalar1=iotapPf[:, 0:1], scalar2=wcol,
                op0=mybir.AluOpType.is_equal, op1=mybir.AluOpType.mult)
            return M16

        # ---------------- layer 1 ----------------
        # AllGather halves emitted INSIDE the loop right after the supertile
        # that completes each half of h1loc, so the Pool sequencer reaches them
        # without draining the whole layer-1 queue. Width-sliced (64-of-128)
        # APs write straight into the padded-row table layout the gather needs.
        hs2 = cfg.shard // 2
        ag_after = {}
        for half, (a, b), tab in ((0, (0, hs2), h1tab0), (1, (hs2, 2 * hs2), h1tab1)):
            last_tile = (b - 1) // P
            st_of = last_tile // cfg.st_tiles
            ag_after[st_of] = ((a, b), tab)

        def emit_ag(st):
            if st not in ag_after:
                return
            (a, b), tab = ag_after.pop(st)
            nc.gpsimd.collective_compute(
                "AllGather", mybir.AluOpType.bypass,
                replica_groups=[list(range(cfg.n_cores))],
                ins=[h1loc[a:b, :].opt()],
                outs=[tab[:].opt()],
            )

        with tc.tile_pool(name="l1s", bufs=3) as l1s, \
             tc.tile_pool(name="l1p", bufs=2, space="PSUM") as l1p:
            for st in range(cfg.n_st):
                tiles = cfg.tiles_of_st(st)
                colstart = st_meta[st][0][0]
                colend = st_meta[st][cfg.src_chunks - 1][0] + st_meta[st][cfg.src_chunks - 1][1]
                n_stc = colend - colstart

                xs_st = l1s.tile([P, n_stc, F_IN], dt.float16, tag="xs")
                nc.sync.dma_start(xs_st[:], xs_in[:, colstart:colend, :])

                for t in tiles:
                    chunks = tile_chunks[t]
                    acc1 = l1p.tile([P, F_IN], dt.float32, tag="acc1", bufs=3)
                    Ms = build_Mdiag(s2nm[:, t:t + 1])
                    nc.tensor.matmul(acc1[:], lhsT=Ms[:], rhs=xown16[:, t, :],
                                     start=True, stop=False)
                    for ci, (col, _st, _c, _jj) in enumerate(chunks):
                        M16 = build_M(ds_all[:, col, 0:1], ds_all[:, col, 1:2],
                                      ds_all[:, col, 3:4], no_pool=True)
                        nc.tensor.matmul(
                            acc1[:], lhsT=M16[:], rhs=xs_st[:, col - colstart, :],
                            start=False, stop=(ci == len(chunks) - 1))
                    # dense: agg[128d,4] -> transpose -> z1[128d,64] -> relu/scale
                    a1n = evac.tile([P, F_IN], dt.float16, tag="a1n")
                    nc.vector.tensor_copy(a1n[:], acc1[:])
                    a1T = l1p.tile([F_IN, P], dt.float16, tag="a1T", bufs=1)
                    nc.tensor.transpose(a1T[:], a1n[:], identP[:])
                    a1s = evac.tile([F_IN, P], dt.float16, tag="a1s")
                    nc.scalar.activation(a1s[:], a1T[:], mybir.ActivationFunctionType.Copy)
                    z1p = l1p.tile([P, F_HID], dt.float32, tag="z1p", bufs=2)
                    nc.tensor.matmul(z1p[:], lhsT=a1s[:], rhs=W1s16[:], start=True, stop=True)
                    t1 = evac.tile([P, F_HID], dt.float16, tag="t1")
                    nc.vector.tensor_tensor(out=t1[:], in0=z1p[:], in1=B1s[:],
                                            op=mybir.AluOpType.add)
                    nc.vector.tensor_scalar(
                        out=h1keep[:, t, :], in0=t1[:], scalar1=snm[:, t:t + 1],
                        scalar2=0.0, op0=mybir.AluOpType.mult, op1=mybir.AluOpType.max)
                    rows = cfg.last_rows if t == cfg.n_tiles - 1 else P
                    nc.sync.dma_start(h1loc[t * P:t * P + rows, 0:F_HID],
                                      h1keep[:rows, t, :])
                emit_ag(st)

        # ---------------- layer 2 ----------------
        with tc.tile_pool(name="l2s", bufs=4) as l2s, \
             tc.tile_pool(name="gpool", bufs=5) as gpool, \
             tc.tile_pool(name="accp", bufs=cfg.n_st) as accp, \
             tc.tile_pool(name="l2p", bufs=1, space="PSUM") as l2p:
            accs = {}

            def emit_chunk_group(st, c):
                tiles = cfg.tiles_of_st(st)
                if c == 0:
                    acc_st = accp.tile([P, len(tiles), F_HID], dt.float16,
                                       tag="accS", name=f"accS{st}")
                    accs[st] = acc_st
                colstart, G, nidx = st_meta[st][c]
                idx_t = l2s.tile([P, G * 8], dt.int16, tag="idx")
                nc.sync.dma_start(idx_t[:], idx_in[:, colstart * 8:(colstart + G) * 8])
                gt = gpool.tile([P, G, F_HID], dt.float16, tag="gath")
                srcs = {0: h1tab0[0:cfg.src_chunk, :],
                        1: h1tab0[cfg.src_chunk:2 * cfg.src_chunk, :],
                        2: h1tab1[0:cfg.src_chunk, :],
                        3: h1tab1[cfg.src_chunk:2 * cfg.src_chunk, :]}
                thin_gather(gt[:], srcs[c][:, 0:F_HID], idx_t[:], nidx)
                for ti, t in enumerate(tiles):
                    chs = [e for e in tile_chunks[t] if e[2] == c]
                    acc = l2p.tile([P, F_HID], dt.float32, tag="accq", bufs=4)
                    if c == 0:
                        Ms = build_Mdiag(snm[:, t:t + 1])
                        nc.tensor.matmul(acc[:], lhsT=Ms[:], rhs=h1keep[:, t, :],
                                         start=True, stop=False)
                    for ci, (col, _st, _c, jj) in enumerate(chs):
                        M16 = build_M(ds_all[:, col, 0:1], ds_all[:, col, 2:3],
                                      ds_all[:, col, 4:5])
                        nc.tensor.matmul(
                            acc[:], lhsT=M16[:], rhs=gt[:, jj, :],
                            start=(ci == 0 and c != 0), stop=(ci == len(chs) - 1),
                        )
                    sl = accs[st][:, ti, :]
                    if c == 0:
                        nc.scalar.activation(sl, acc[:], mybir.ActivationFunctionType.Copy)
                    else:
                        nc.vector.tensor_add(sl, acc[:], sl)

            def emit_final(st):
                tiles = cfg.tiles_of_st(st)
                for ti, t in enumerate(tiles):
                    sl = accs[st][:, ti, :]
                    aT = l2p.tile([F_HID, P], dt.float16, tag="aT", bufs=1)
                    nc.tensor.transpose(aT[:], sl, identP[:])
                    a2s = evac.tile([F_HID, P], dt.float16, tag="a2s")
                    nc.scalar.activation(a2s[:], aT[:], mybir.ActivationFunctionType.Copy)
                    ph2 = l2p.tile([F_HID, P], dt.float32, tag="ph2", bufs=2)
                    nc.tensor.matmul(ph2[:], lhsT=W2s16[:], rhs=a2s[:], start=True, stop=True)
                    h2f = evac.tile([F_HID, P], dt.float16, tag="h2f")
                    nc.scalar.activation(h2f[:], ph2[:], mybir.ActivationFunctionType.Relu,
                                         bias=b2s[:, 0:1])
                    po = l2p.tile([F_OUT, P], dt.float32, tag="po", bufs=1)
                    nc.tensor.matmul(po[:], lhsT=Wfs16[:], rhs=h2f[:], start=True, stop=True)
                    osb = evac.tile([F_OUT, P], dt.float32, tag="osb")
                    nc.scalar.activation(osb[:], po[:], mybir.ActivationFunctionType.Identity,
                                         bias=bfs[:, 0:1])
                    rows = cfg.last_rows if t == cfg.n_tiles - 1 else P
                    nc.sync.dma_start(out_ext[:, t * P:t * P + rows], osb[:, :rows])

            for st in range(cfg.n_st):
                emit_chunk_group(st, 0)
                emit_chunk_group(st, 1)
            for st in range(cfg.n_st):
                emit_chunk_group(st, 2)
                emit_chunk_group(st, 3)
                emit_final(st)

    nc.finalize()
    return nc


def make_in_maps(cfg, dev, wb):
    maps = []
    for cpu in range(cfg.n_cores):
        d = dev[cpu]
        maps.append({
            "xs": d["xs"], "ds": d["ds"], "idx": d["idx"],
            "s_nm": d["s_nm"], "s2_nm": d["s2_nm"], "xown": d["xown"],
            **{k: wb[k] for k in ("W1", "W2", "Wf", "B1", "b2", "bf")},
        })
    return maps


def kernel(x, edge_index, W1, b1, W2, b2, Wf, bf, _trace=False, _tmpdir=None):
    from concourse.bass_utils import run_bass_kernel_spmd

    cfg = CFG
    dev, wb, sched = preprocess(cfg, x, edge_index, W1, b1, W2, b2, Wf, bf)
    nc = build(cfg, sched)
    in_maps = make_in_maps(cfg, dev, wb)
    res = run_bass_kernel_spmd(nc, in_maps, core_ids=list(range(cfg.n_cores)),
                               trace=_trace, tmpdir=_tmpdir)
    out = np.concatenate([res.results[c]["out_fm"].T for c in range(cfg.n_cores)], axis=0)
    kernel._last_results = res
    return out.astype(np.float32)



# revision 5
# speedup vs baseline: 1.2669x; 1.2669x over previous
"""Trainium2 Bass kernel for a 2-layer GCN (GCNConv -> relu -> GCNConv -> relu -> Linear).

Math: with s = deg^-1/2 (deg over dst incl. self-loops):
  h1 = relu( s_d * (A_ind @ (s_s * x)) @ W1 + b1 )   (aggregate 4-wide first)
  h2 = relu( W2 @ (s_d * (A_ind @ h1')) + b2 ),  h1' = h1 * s  (table prescale)
  out = h2 @ Wf + bf
A_ind is the pure 0/1 edge indicator: s_src is folded into the streamed
features (host), s_dst applied per dst-tile after aggregation.

Device strategy (8 cores, nodes sharded by dst, SPMD one program):
  - STRUCTURAL SLOTS: each dst gets a fixed quota of edge slots (L1: 32 per
    tile; L2: 8 per (dst, src_chunk)); a 128-slot chunk covers 16 consecutive
    dst x 8 slots, aggregated with a CONSTANT block-diagonal selection matrix
    Mst[:, j, :] (one per 16-dst slice, host-uploaded) -- no per-chunk M build.
    Overflow edges beyond the quota go through classic built-M chunks
    (DVE is_equal vs iota), but those are now rare (~15% of chunks).
  - slot padding: L1 pad slots stream zero features; L2 pad slots gather a
    reserved zero row (row 0 of each 25001-row table chunk, idx 0; real rows
    at idx 1+trow%25000).
  - AllGather moves only the 64 real fp16 feature columns (strided out AP into
    the 256B-row padded gather table) in 2 halves emitted inside the L1 loop.
  - layer-2 source rows fetched per-slot via raw InstDMAGatherAnt (elem_size
    64, 256B stride -> 128B payload/descriptor).
"""
import numpy as np
from contextlib import ExitStack
from dataclasses import dataclass


@dataclass(frozen=True)
class Cfg:
    n_nodes: int = 100000
    n_cores: int = 8
    f_in: int = 4
    f_hid: int = 64
    f_out: int = 2
    src_chunks: int = 4
    st_tiles: int = 4
    q1: int = 32   # L1 structural slots per dst (4 chunks x 8)
    q2: int = 8    # L2 structural slots per (dst, src_chunk) (1 chunk x 8)

    @property
    def shard(self):
        return self.n_nodes // self.n_cores

    @property
    def n_tiles(self):
        return (self.shard + 127) // 128

    @property
    def last_rows(self):
        return self.shard - (self.n_tiles - 1) * 128

    @property
    def n_st(self):
        return (self.n_tiles + self.st_tiles - 1) // self.st_tiles

    def tiles_of_st(self, st):
        return list(range(st * self.st_tiles, min((st + 1) * self.st_tiles, self.n_tiles)))

    def rows_of_tile(self, t):
        return self.last_rows if t == self.n_tiles - 1 else 128

    def slices_of_tile(self, t):
        return (self.rows_of_tile(t) + 15) // 16


CFG = Cfg()
P = 128
FP = 128   # fp16 table row padded to 256B (dma_gather min stride)
S = 8      # slots per dst per structural chunk
HS = CFG.shard // 2          # rows per AllGather half per core
CJ = HS * CFG.n_cores // 2   # 25000 rows per table idx chunk


def _ranks(sorted_keys):
    """rank of each element within its run of equal (sorted) keys."""
    n = len(sorted_keys)
    if n == 0:
        return np.zeros(0, dtype=np.int64)
    first = np.empty(n, dtype=bool)
    first[0] = True
    np.not_equal(sorted_keys[1:], sorted_keys[:-1], out=first[1:])
    gstart = np.flatnonzero(first)
    gid = np.cumsum(first) - 1
    return np.arange(n) - gstart[gid]


def preprocess(cfg, x, edge_index, W1, b1, W2, b2, Wf, bf):
    """Host-side sharding: slot assignment, overflow grouping, streams."""
    F_IN = cfg.f_in
    src0 = np.asarray(edge_index[0], dtype=np.int64)
    dst0 = np.asarray(edge_index[1], dtype=np.int64)
    deg = (np.bincount(dst0, minlength=cfg.n_nodes) + 1).astype(np.float64)
    s = (1.0 / np.sqrt(deg)).astype(np.float32)

    # self loops appended as ordinary edges
    loop = np.arange(cfg.n_nodes, dtype=np.int64)
    src = np.concatenate([src0, loop])
    dst = np.concatenate([dst0, loop])
    x = np.asarray(x, dtype=np.float32)
    xsrc_all = (x * s[:, None]).astype(np.float16)  # prescaled by s_src

    core_id = dst // cfg.shard

    # L2 table mapping for each global src node
    lr_all = src % cfg.shard
    score_all = src // cfg.shard
    h_all = lr_all // HS
    trow_all = score_all * HS + (lr_all - h_all * HS)
    c_all = h_all * 2 + trow_all // CJ          # src chunk 0..3
    idx_all = 1 + (trow_all % CJ)               # 1-based; 0 = zero row

    n_tiles, n_st = cfg.n_tiles, cfg.n_st

    # ---- pass 1: per-core sorted edge views + overflow counts ----
    cores = []
    ovf1 = np.zeros((cfg.n_cores, n_tiles), dtype=np.int64)
    ovf2 = np.zeros((cfg.n_cores, cfg.src_chunks * n_tiles), dtype=np.int64)
    for cpu in range(cfg.n_cores):
        m = core_id == cpu
        sc, dc = src[m], dst[m]
        dl = dc - cpu * cfg.shard
        tl = dl // P
        d128 = dl % P
        cch = c_all[m]
        idxv = idx_all[m]

        # L1: sort by (t, d128)
        o1 = np.lexsort((d128, tl))
        t1, dd1, s1v = tl[o1], d128[o1], sc[o1]
        r1 = _ranks(t1 * P + dd1)
        m1o = r1 >= cfg.q1
        ovf1[cpu] = np.bincount(t1[m1o], minlength=n_tiles)

        # L2: sort by (c, t, d128)
        o2 = np.lexsort((d128, tl, cch))
        c2, t2, dd2, i2 = cch[o2], tl[o2], d128[o2], idxv[o2]
        r2 = _ranks((c2 * n_tiles + t2) * P + dd2)
        m2o = r2 >= cfg.q2
        ovf2[cpu] = np.bincount((c2 * n_tiles + t2)[m2o],
                                minlength=cfg.src_chunks * n_tiles)
        cores.append((o1, t1, dd1, s1v, r1, o2, c2, t2, dd2, i2, r2, sc, dc))

    C1 = np.maximum(1, -(-ovf1.max(axis=0) // P))            # [n_tiles]
    C2 = np.maximum(1, -(-ovf2.max(axis=0) // P)).reshape(cfg.src_chunks, n_tiles)

    # ---- shared column layouts ----
    # L1: per st: for t: nS1(t) structural cols then C1(t) ovf cols
    nS1 = [4 * cfg.slices_of_tile(t) for t in range(n_tiles)]
    col1_struct = np.zeros(n_tiles, dtype=np.int64)   # base of structural cols
    col1_ovf = np.zeros(n_tiles, dtype=np.int64)
    tile_chunks1 = [[] for _ in range(n_tiles)]       # (col, kind, j, ovfidx)
    l1_st_cols = []
    ovfidx1 = np.zeros(n_tiles, dtype=np.int64)
    nc1 = 0
    nov1 = 0
    for st in range(n_st):
        st_start = nc1
        for t in cfg.tiles_of_st(st):
            col1_struct[t] = nc1
            for j in range(cfg.slices_of_tile(t)):
                for k in range(4):
                    tile_chunks1[t].append((nc1, 's', j, -1))
                    nc1 += 1
            col1_ovf[t] = nc1
            ovfidx1[t] = nov1
            for q in range(int(C1[t])):
                tile_chunks1[t].append((nc1, 'o', -1, nov1))
                nc1 += 1
                nov1 += 1
        l1_st_cols.append((st_start, nc1))
    NC1, NOV1 = nc1, nov1

    # L2: per (st, c): for t: q2//S structural cols + C2(c,t) ovf cols
    col2_struct = np.zeros((cfg.src_chunks, n_tiles), dtype=np.int64)
    col2_ovf = np.zeros((cfg.src_chunks, n_tiles), dtype=np.int64)
    ovfidx2 = np.zeros((cfg.src_chunks, n_tiles), dtype=np.int64)
    tile_chunks2 = {}                                 # (c,t) -> list
    st_meta2 = [[None] * cfg.src_chunks for _ in range(n_st)]
    nc2 = 0
    nov2 = 0
    for st in range(n_st):
        for c in range(cfg.src_chunks):
            colstart = nc2
            for t in cfg.tiles_of_st(st):
                lst = []
                col2_struct[c, t] = nc2
                for j in range(cfg.slices_of_tile(t)):
                    lst.append((nc2, 's', j, -1))
                    nc2 += 1
                col2_ovf[c, t] = nc2
                ovfidx2[c, t] = nov2
                for q in range(int(C2[c, t])):
                    lst.append((nc2, 'o', -1, nov2))
                    nc2 += 1
                    nov2 += 1
                tile_chunks2[(c, t)] = lst
            G = nc2 - colstart
            st_meta2[st][c] = (colstart, G, G * P)
    NC2, NOV2 = nc2, nov2

    # ---- pass 2: per-core device arrays ----
    dev = []
    for cpu in range(cfg.n_cores):
        (o1, t1, dd1, s1v, r1, o2, c2, t2, dd2, i2, r2, sc, dc) = cores[cpu]

        # L1 stream positions
        j1 = dd1 // 16
        p16_1 = dd1 % 16
        ms = r1 < cfg.q1
        col_s = col1_struct[t1[ms]] + j1[ms] * 4 + r1[ms] // S
        row_s = p16_1[ms] * S + r1[ms] % S
        pos_s = col_s * P + row_s
        mo = ~ms
        to = t1[mo]
        orank = _ranks(to)  # overflow edges sorted by t already
        col_o = col1_ovf[to] + orank // P
        row_o = orank % P
        pos_o = col_o * P + row_o

        xs1 = np.zeros((NC1 * P, F_IN), dtype=np.float16)
        xs1[pos_s] = xsrc_all[s1v[ms]]
        xs1[pos_o] = xsrc_all[s1v[mo]]
        dv1 = np.full(NOV1 * P, -1.0, dtype=np.float32)
        dv1[(ovfidx1[to] + orank // P) * P + row_o] = dd1[mo].astype(np.float32)

        xs = np.ascontiguousarray(xs1.reshape(NC1, P, F_IN).transpose(1, 0, 2))
        dv1w = np.ascontiguousarray(dv1.reshape(NOV1, P).T)

        # L2 stream positions
        j2 = dd2 // 16
        p16_2 = dd2 % 16
        ms2 = r2 < cfg.q2
        colb = col2_struct[c2[ms2], t2[ms2]] + j2[ms2]
        rowb = p16_2[ms2] * S + r2[ms2]
        pos2s = colb * P + rowb
        mo2 = ~ms2
        key_o2 = c2[mo2] * n_tiles + t2[mo2]
        orank2 = _ranks(key_o2)
        col_o2 = col2_ovf[c2[mo2], t2[mo2]] + orank2 // P
        row_o2 = orank2 % P
        pos2o = col_o2 * P + row_o2

        idx2 = np.zeros(NC2 * P, dtype=np.int16)
        idx2[pos2s] = i2[ms2].astype(np.int16)
        idx2[pos2o] = i2[mo2].astype(np.int16)
        dv2 = np.full(NOV2 * P, -1.0, dtype=np.float32)
        dv2[(ovfidx2[c2[mo2], t2[mo2]] + orank2 // P) * P + row_o2] = \
            dd2[mo2].astype(np.float32)

        idx_w = np.tile(idx2.reshape(NC2 * 8, 16).T, (8, 1))
        dv2w = np.ascontiguousarray(dv2.reshape(NOV2, P).T)

        s_core = np.zeros(n_tiles * P, dtype=np.float32)
        s_core[:cfg.shard] = s[cpu * cfg.shard:(cpu + 1) * cfg.shard]
        s_nm = s_core.reshape(n_tiles, P).T.copy()

        dev.append(dict(xs=xs, dv1=dv1w, idx=np.ascontiguousarray(idx_w),
                        dv2=dv2w, s_nm=s_nm))

    # structural selection matrices: Mst[r, j, c] = 1 if c == 16j + r//8
    Mst = np.zeros((P, 8, P), dtype=np.float16)
    r = np.arange(P)
    for j in range(8):
        Mst[r, j, 16 * j + r // S] = 1.0

    wb = dict(
        W1=np.asarray(W1, np.float32), W2=np.asarray(W2, np.float32),
        Wf=np.asarray(Wf, np.float32),
        B1=np.broadcast_to(np.asarray(b1, np.float32).reshape(1, cfg.f_hid),
                           (P, cfg.f_hid)).copy(),
        b2=np.asarray(b2, np.float32).reshape(cfg.f_hid, 1),
        bf=np.asarray(bf, np.float32).reshape(cfg.f_out, 1),
        Mst=Mst,
    )
    sched = dict(NC1=NC1, NOV1=NOV1, NC2=NC2, NOV2=NOV2,
                 l1_st_cols=l1_st_cols, tile_chunks1=tile_chunks1,
                 st_meta2=st_meta2, tile_chunks2=tile_chunks2)
    return dev, wb, sched


def build(cfg, sched):
    import concourse.bass as bass
    import concourse.mybir as mybir
    import concourse.tile as tile
    from concourse import bacc

    dt = mybir.dt
    F_IN, F_HID, F_OUT = cfg.f_in, cfg.f_hid, cfg.f_out
    NC1, NOV1 = sched["NC1"], sched["NOV1"]
    NC2, NOV2 = sched["NC2"], sched["NOV2"]
    l1_st_cols = sched["l1_st_cols"]
    tile_chunks1 = sched["tile_chunks1"]
    st_meta2 = sched["st_meta2"]
    tile_chunks2 = sched["tile_chunks2"]

    nc = bacc.Bacc("TRN2", target_bir_lowering=False, num_devices=cfg.n_cores)
    xs_in = nc.declare_dram_parameter("xs", [P, NC1, F_IN], dt.float16, isOutput=False)
    dv1_in = nc.declare_dram_parameter("dv1", [P, NOV1], dt.float32, isOutput=False)
    idx_in = nc.declare_dram_parameter("idx", [P, NC2 * 8], dt.int16, isOutput=False)
    dv2_in = nc.declare_dram_parameter("dv2", [P, NOV2], dt.float32, isOutput=False)
    snm_in = nc.declare_dram_parameter("s_nm", [P, cfg.n_tiles], dt.float32, isOutput=False)
    Mst_in = nc.declare_dram_parameter("Mst", [P, 8, P], dt.float16, isOutput=False)
    W1_in = nc.declare_dram_parameter("W1", [F_IN, F_HID], dt.float32, isOutput=False)
    W2_in = nc.declare_dram_parameter("W2", [F_HID, F_HID], dt.float32, isOutput=False)
    Wf_in = nc.declare_dram_parameter("Wf", [F_HID, F_OUT], dt.float32, isOutput=False)
    B1_in = nc.declare_dram_parameter("B1", [P, F_HID], dt.float32, isOutput=False)
    b2_in = nc.declare_dram_parameter("b2", [F_HID, 1], dt.float32, isOutput=False)
    bf_in = nc.declare_dram_parameter("bf", [F_OUT, 1], dt.float32, isOutput=False)
    out_ext = nc.declare_dram_parameter("out_fm", [F_OUT, cfg.shard], dt.float32, isOutput=True)

    def thin_gather(out_ap, in_ap, idxs_ap, num_idxs):
        """dma_gather fetching the first 128B of each 256B-strided table row."""
        eng = nc.gpsimd
        _in_ap = eng.lower_ap_dma(in_ap, for_custom_bir_dma=True)
        _idxs_ap = eng.lower_ap(idxs_ap)
        _out_ap = eng.lower_ap(out_ap)
        return eng.add_instruction(
            mybir.InstDMAGatherAnt(
                name=eng.bass.get_next_instruction_name(),
                ins=[*_in_ap, _idxs_ap, eng.lower_val_access(eng.to_reg(num_idxs))],
                outs=[_out_ap],
                transpose=False,
                num_idxs=num_idxs,
                elem_size=F_HID,
                stride_bytes_256=1,
                gen_mode=0,
                single_packet=False,
                queue_num=0,
                sbuf_tokens_per_rank=0,
                sbuf_free_dim_per_rank=0,
                sbuf_free_dim_pad_per_rank=0,
                sbuf_byte_offset=0,
            )
        )

    with tile.TileContext(nc, num_cores=cfg.n_cores) as tc, ExitStack() as ctx:
        dram = ctx.enter_context(tc.tile_pool(name="dram", bufs=1, space="DRAM"))
        const = ctx.enter_context(tc.tile_pool(name="const", bufs=1))
        mpool = ctx.enter_context(tc.tile_pool(name="mpool", bufs=12))
        evac = ctx.enter_context(tc.tile_pool(name="evac", bufs=6))

        h1loc = dram.tile([cfg.shard, F_HID], dt.float16)
        # gather tables: 2 halves x [2 chunks, 1 zero row + 25000 rows, FP]
        h1tab0 = dram.tile([2, CJ + 1, FP], dt.float16, name="h1tab0")
        h1tab1 = dram.tile([2, CJ + 1, FP], dt.float16, name="h1tab1")
        # compact AllGather landing buffers (collective outs must be contiguous)
        h1cmp0 = dram.tile([2, CJ, F_HID], dt.float16, name="h1cmp0")
        h1cmp1 = dram.tile([2, CJ, F_HID], dt.float16, name="h1cmp1")

        iota_i = const.tile([P, P], dt.int16)
        nc.gpsimd.iota(iota_i[:], pattern=[[1, P]], base=0, channel_multiplier=0)
        iota16 = const.tile([P, P], dt.float16)
        nc.vector.tensor_copy(iota16[:], iota_i[:])
        iotapP = const.tile([P, 1], dt.int16)
        nc.gpsimd.iota(iotapP[:], pattern=[[0, 1]], base=0, channel_multiplier=1)
        iotapPf = const.tile([P, 1], dt.float32)
        nc.vector.tensor_copy(iotapPf[:], iotapP[:])
        identP = const.tile([P, P], dt.float16)
        nc.vector.tensor_scalar(out=identP[:], in0=iota16[:], scalar1=iotapPf[:, 0:1],
                                scalar2=None, op0=mybir.AluOpType.is_equal)
        zrow = const.tile([2, FP], dt.float16)
        nc.vector.memset(zrow[:], 0.0)
        for half in range(2):
            nc.sync.dma_start(h1tab0[half, 0:1, :], zrow[half:half + 1, :])
            nc.sync.dma_start(h1tab1[half, 0:1, :], zrow[half:half + 1, :])

        W1s = const.tile([F_IN, F_HID], dt.float32)
        W2s = const.tile([F_HID, F_HID], dt.float32)
        Wfs = const.tile([F_HID, F_OUT], dt.float32)
        B1s = const.tile([P, F_HID], dt.float32)
        b2s = const.tile([F_HID, 1], dt.float32)
        bfs = const.tile([F_OUT, 1], dt.float32)
        snm = const.tile([P, cfg.n_tiles], dt.float32)
        Msts = const.tile([P, 8, P], dt.float16)
        dv1s = const.tile([P, NOV1], dt.float32)
        dv2s = const.tile([P, NOV2], dt.float32)
        nc.sync.dma_start(W1s[:], W1_in[:])
        nc.sync.dma_start(W2s[:], W2_in[:])
        nc.sync.dma_start(Wfs[:], Wf_in[:])
        nc.sync.dma_start(B1s[:], B1_in[:])
        nc.sync.dma_start(b2s[:], b2_in[:])
        nc.sync.dma_start(bfs[:], bf_in[:])
        nc.sync.dma_start(snm[:], snm_in[:])
        nc.sync.dma_start(Msts[:], Mst_in[:])
        nc.sync.dma_start(dv1s[:], dv1_in[:])
        nc.sync.dma_start(dv2s[:], dv2_in[:])

        W1s16 = const.tile([F_IN, F_HID], dt.float16)
        nc.scalar.activation(W1s16[:], W1s[:], mybir.ActivationFunctionType.Copy)
        W2s16 = const.tile([F_HID, F_HID], dt.float16)
        nc.scalar.activation(W2s16[:], W2s[:], mybir.ActivationFunctionType.Copy)
        Wfs16 = const.tile([F_HID, F_OUT], dt.float16)
        nc.scalar.activation(Wfs16[:], Wfs[:], mybir.ActivationFunctionType.Copy)

        def build_M(scol):
            """Indicator matrix [128e, 128d] = (iota == dstv) on DVE."""
            M16 = mpool.tile([P, P], dt.float16, tag="M")
            nc.vector.tensor_scalar(
                out=M16[:], in0=iota16[:], scalar1=scol, scalar2=None,
                op0=mybir.AluOpType.is_equal)
            return M16

        # ---------------- layer 1 ----------------
        hs2 = cfg.shard // 2
        ag_after = {}
        for half, (a, b), tab, cmp_ in ((0, (0, hs2), h1tab0, h1cmp0),
                                        (1, (hs2, 2 * hs2), h1tab1, h1cmp1)):
            last_tile = (b - 1) // P
            st_of = last_tile // cfg.st_tiles
            ag_after[st_of] = ((a, b), tab, cmp_)

        def emit_ag(st):
            if st not in ag_after:
                return
            (a, b), tab, cmp_ = ag_after.pop(st)
            nc.gpsimd.collective_compute(
                "AllGather", mybir.AluOpType.bypass,
                replica_groups=[list(range(cfg.n_cores))],
                ins=[h1loc[a:b, :].opt()],
                outs=[cmp_[:].opt()],
            )
            # expand compact rows into the 256B-strided gather table
            nc.sync.dma_start(tab[:, 1:, 0:F_HID], cmp_[:])

        with tc.tile_pool(name="l1s", bufs=3) as l1s, \
             tc.tile_pool(name="l1p", bufs=2, space="PSUM") as l1p:
            for st in range(cfg.n_st):
                colstart, colend = l1_st_cols[st]
                n_stc = colend - colstart
                xs_st = l1s.tile([P, n_stc, F_IN], dt.float16, tag="xs")
                nc.sync.dma_start(xs_st[:], xs_in[:, colstart:colend, :])

                for t in cfg.tiles_of_st(st):
                    chunks = tile_chunks1[t]
                    acc1 = l1p.tile([P, F_IN], dt.float32, tag="acc1", bufs=3)
                    for ci, (col, kind, j, oidx) in enumerate(chunks):
                        lhs = Msts[:, j, :] if kind == 's' else \
                            build_M(dv1s[:, oidx:oidx + 1])[:]
                        nc.tensor.matmul(
                            acc1[:], lhsT=lhs, rhs=xs_st[:, col - colstart, :],
                            start=(ci == 0), stop=(ci == len(chunks) - 1))
                    # dense: scale by s_dst -> transpose -> W1 -> +b1, relu, *s
                    a1n = evac.tile([P, F_IN], dt.float16, tag="a1n")
                    nc.vector.tensor_scalar(out=a1n[:], in0=acc1[:],
                                            scalar1=snm[:, t:t + 1], scalar2=None,
                                            op0=mybir.AluOpType.mult)
                    a1T = l1p.tile([F_IN, P], dt.float16, tag="a1T", bufs=1)
                    nc.tensor.transpose(a1T[:], a1n[:], identP[:])
                    a1s = evac.tile([F_IN, P], dt.float16, tag="a1s")
                    nc.scalar.activation(a1s[:], a1T[:], mybir.ActivationFunctionType.Copy)
                    z1p = l1p.tile([P, F_HID], dt.float32, tag="z1p", bufs=2)
                    nc.tensor.matmul(z1p[:], lhsT=a1s[:], rhs=W1s16[:], start=True, stop=True)
                    t1 = evac.tile([P, F_HID], dt.float16, tag="t1")
                    nc.vector.tensor_tensor(out=t1[:], in0=z1p[:], in1=B1s[:],
                                            op=mybir.AluOpType.add)
                    h1k = evac.tile([P, F_HID], dt.float16, tag="h1k")
                    nc.vector.tensor_scalar(
                        out=h1k[:], in0=t1[:], scalar1=snm[:, t:t + 1],
                        scalar2=0.0, op0=mybir.AluOpType.mult, op1=mybir.AluOpType.max)
                    rows = cfg.rows_of_tile(t)
                    nc.sync.dma_start(h1loc[t * P:t * P + rows, :], h1k[:rows, :])
                emit_ag(st)

        # ---------------- layer 2 ----------------
        with tc.tile_pool(name="l2s", bufs=4) as l2s, \
             tc.tile_pool(name="gpool", bufs=5) as gpool, \
             tc.tile_pool(name="accp", bufs=cfg.n_st) as accp, \
             tc.tile_pool(name="l2p", bufs=1, space="PSUM") as l2p:
            accs = {}
            srcs = {0: h1tab0[0], 1: h1tab0[1], 2: h1tab1[0], 3: h1tab1[1]}

            def emit_chunk_group(st, c):
                tiles = cfg.tiles_of_st(st)
                if c == 0:
                    acc_st = accp.tile([P, len(tiles), F_HID], dt.float16,
                                       tag="accS", name=f"accS{st}")
                    accs[st] = acc_st
                colstart, G, nidx = st_meta2[st][c]
                idx_t = l2s.tile([P, G * 8], dt.int16, tag="idx")
                nc.sync.dma_start(idx_t[:], idx_in[:, colstart * 8:(colstart + G) * 8])
                gt = gpool.tile([P, G, F_HID], dt.float16, tag="gath")
                thin_gather(gt[:], srcs[c][:, 0:F_HID], idx_t[:], nidx)
                for ti, t in enumerate(tiles):
                    chs = tile_chunks2[(c, t)]
                    acc = l2p.tile([P, F_HID], dt.float32, tag="accq", bufs=4)
                    for ci, (col, kind, j, oidx) in enumerate(chs):
                        lhs = Msts[:, j, :] if kind == 's' else \
                            build_M(dv2s[:, oidx:oidx + 1])[:]
                        nc.tensor.matmul(
                            acc[:], lhsT=lhs, rhs=gt[:, col - colstart, :],
                            start=(ci == 0), stop=(ci == len(chs) - 1))
                    sl = accs[st][:, ti, :]
                    if c == 0:
                        nc.scalar.activation(sl, acc[:], mybir.ActivationFunctionType.Copy)
                    else:
                        nc.vector.tensor_add(sl, acc[:], sl)

            def emit_final(st):
                tiles = cfg.tiles_of_st(st)
                for ti, t in enumerate(tiles):
                    sl = accs[st][:, ti, :]
                    slf = evac.tile([P, F_HID], dt.float16, tag="slf")
                    nc.vector.tensor_scalar(out=slf[:], in0=sl, scalar1=snm[:, t:t + 1],
                                            scalar2=None, op0=mybir.AluOpType.mult)
                    aT = l2p.tile([F_HID, P], dt.float16, tag="aT", bufs=1)
                    nc.tensor.transpose(aT[:], slf[:], identP[:])
                    a2s = evac.tile([F_HID, P], dt.float16, tag="a2s")
                    nc.scalar.activation(a2s[:], aT[:], mybir.ActivationFunctionType.Copy)
                    ph2 = l2p.tile([F_HID, P], dt.float32, tag="ph2", bufs=2)
                    nc.tensor.matmul(ph2[:], lhsT=W2s16[:], rhs=a2s[:], start=True, stop=True)
                    h2f = evac.tile([F_HID, P], dt.float16, tag="h2f")
                    nc.scalar.activation(h2f[:], ph2[:], mybir.ActivationFunctionType.Relu,
                                         bias=b2s[:, 0:1])
                    po = l2p.tile([F_OUT, P], dt.float32, tag="po", bufs=1)
                    nc.tensor.matmul(po[:], lhsT=Wfs16[:], rhs=h2f[:], start=True, stop=True)
                    osb = evac.tile([F_OUT, P], dt.float32, tag="osb")
                    nc.scalar.activation(osb[:], po[:], mybir.ActivationFunctionType.Identity,
                                         bias=bfs[:, 0:1])
                    rows = cfg.rows_of_tile(t)
                    nc.sync.dma_start(out_ext[:, t * P:t * P + rows], osb[:, :rows])

            for st in range(cfg.n_st):
                emit_chunk_group(st, 0)
                emit_chunk_group(st, 1)
            for st in range(cfg.n_st):
                emit_chunk_group(st, 2)
                emit_chunk_group(st, 3)
                emit_final(st)

    nc.finalize()
    return nc


def make_in_maps(cfg, dev, wb):
    maps = []
    for cpu in range(cfg.n_cores):
        d = dev[cpu]
        maps.append({
            "xs": d["xs"], "dv1": d["dv1"], "idx": d["idx"], "dv2": d["dv2"],
            "s_nm": d["s_nm"],
            **{k: wb[k] for k in ("W1", "W2", "Wf", "B1", "b2", "bf", "Mst")},
        })
    return maps


def kernel(x, edge_index, W1, b1, W2, b2, Wf, bf, _trace=False, _tmpdir=None):
    from concourse.bass_utils import run_bass_kernel_spmd

    cfg = CFG
    dev, wb, sched = preprocess(cfg, x, edge_index, W1, b1, W2, b2, Wf, bf)
    nc = build(cfg, sched)
    in_maps = make_in_maps(cfg, dev, wb)
    res = run_bass_kernel_spmd(nc, in_maps, core_ids=list(range(cfg.n_cores)),
                               trace=_trace, tmpdir=_tmpdir)
    out = np.concatenate([res.results[c]["out_fm"].T for c in range(cfg.n_cores)], axis=0)
    kernel._last_results = res
    return out.astype(np.float32)


# revision 7
# speedup vs baseline: 1.2801x; 1.0104x over previous
"""Trainium2 Bass kernel for a 2-layer GCN (GCNConv -> relu -> GCNConv -> relu -> Linear).

Math: with s = deg^-1/2 (deg over dst incl. self-loops):
  h1 = relu( s_d * (A_ind @ (s_s * x)) @ W1 + b1 )   (aggregate 4-wide first)
  h2 = relu( W2 @ (s_d * (A_ind @ h1')) + b2 ),  h1' = h1 * s  (table prescale)
  out = h2 @ Wf + bf
A_ind is the pure 0/1 edge indicator: s_src is folded into the streamed
features (host), s_dst applied per dst-tile after aggregation.

Device strategy (8 cores, nodes sharded by dst, SPMD one program):
  - STRUCTURAL SLOTS: each dst gets a fixed quota of edge slots (L1: 32 per
    tile; L2: 8 per (dst, src_chunk)); a 128-slot chunk covers 16 consecutive
    dst x 8 slots, aggregated with a CONSTANT block-diagonal selection matrix
    Mst[:, j, :] (one per 16-dst slice, host-uploaded) -- no per-chunk M build.
    Overflow edges beyond the quota go through classic built-M chunks
    (DVE is_equal vs iota), but those are now rare (~15% of chunks).
  - slot padding: L1 pad slots stream zero features; L2 pad slots gather a
    reserved zero row (row 0 of each 25001-row table chunk, idx 0; real rows
    at idx 1+trow%25000).
  - AllGather moves only the 64 real fp16 feature columns (strided out AP into
    the 256B-row padded gather table) in 2 halves emitted inside the L1 loop.
  - layer-2 source rows fetched per-slot via raw InstDMAGatherAnt (elem_size
    64, 256B stride -> 128B payload/descriptor).
"""
import numpy as np
from contextlib import ExitStack
from dataclasses import dataclass


@dataclass(frozen=True)
class Cfg:
    n_nodes: int = 100000
    n_cores: int = 8
    f_in: int = 4
    f_hid: int = 64
    f_out: int = 2
    src_chunks: int = 4
    st_tiles: int = 4
    q1: int = 32   # L1 structural slots per dst (4 chunks x 8)
    q2: int = 8    # L2 structural slots per (dst, src_chunk) (1 chunk x 8)

    @property
    def shard(self):
        return self.n_nodes // self.n_cores

    @property
    def n_tiles(self):
        return (self.shard + 127) // 128

    @property
    def last_rows(self):
        return self.shard - (self.n_tiles - 1) * 128

    @property
    def n_st(self):
        return (self.n_tiles + self.st_tiles - 1) // self.st_tiles

    def tiles_of_st(self, st):
        return list(range(st * self.st_tiles, min((st + 1) * self.st_tiles, self.n_tiles)))

    def rows_of_tile(self, t):
        return self.last_rows if t == self.n_tiles - 1 else 128

    def slices_of_tile(self, t):
        return (self.rows_of_tile(t) + 15) // 16


CFG = Cfg()
P = 128
FP = 128   # fp16 table row padded to 256B (dma_gather min stride)
S = 8      # slots per dst per structural chunk
HS = CFG.shard // 2          # rows per AllGather half per core
CJ = HS * CFG.n_cores // 2   # 25000 rows per table idx chunk


def _ranks(sorted_keys):
    """rank of each element within its run of equal (sorted) keys."""
    n = len(sorted_keys)
    if n == 0:
        return np.zeros(0, dtype=np.int64)
    first = np.empty(n, dtype=bool)
    first[0] = True
    np.not_equal(sorted_keys[1:], sorted_keys[:-1], out=first[1:])
    gstart = np.flatnonzero(first)
    gid = np.cumsum(first) - 1
    return np.arange(n) - gstart[gid]


def preprocess(cfg, x, edge_index, W1, b1, W2, b2, Wf, bf):
    """Host-side sharding: slot assignment, overflow grouping, streams."""
    F_IN = cfg.f_in
    src0 = np.asarray(edge_index[0], dtype=np.int64)
    dst0 = np.asarray(edge_index[1], dtype=np.int64)
    deg = (np.bincount(dst0, minlength=cfg.n_nodes) + 1).astype(np.float64)
    s = (1.0 / np.sqrt(deg)).astype(np.float32)

    # self loops appended as ordinary edges
    loop = np.arange(cfg.n_nodes, dtype=np.int64)
    src = np.concatenate([src0, loop])
    dst = np.concatenate([dst0, loop])
    x = np.asarray(x, dtype=np.float32)
    xsrc_all = (x * s[:, None]).astype(np.float16)  # prescaled by s_src

    core_id = dst // cfg.shard

    # L2 table mapping for each global src node
    lr_all = src % cfg.shard
    score_all = src // cfg.shard
    h_all = lr_all // HS
    trow_all = score_all * HS + (lr_all - h_all * HS)
    c_all = h_all * 2 + trow_all // CJ          # src chunk 0..3
    idx_all = 1 + (trow_all % CJ)               # 1-based; 0 = zero row

    n_tiles, n_st = cfg.n_tiles, cfg.n_st

    # ---- pass 1: per-core sorted edge views + overflow counts ----
    cores = []
    ovf1 = np.zeros((cfg.n_cores, n_tiles), dtype=np.int64)
    ovf2 = np.zeros((cfg.n_cores, cfg.src_chunks * n_tiles), dtype=np.int64)
    for cpu in range(cfg.n_cores):
        m = core_id == cpu
        sc, dc = src[m], dst[m]
        dl = dc - cpu * cfg.shard
        tl = dl // P
        d128 = dl % P
        cch = c_all[m]
        idxv = idx_all[m]

        # L1: sort by (t, d128)
        o1 = np.lexsort((d128, tl))
        t1, dd1, s1v = tl[o1], d128[o1], sc[o1]
        r1 = _ranks(t1 * P + dd1)
        m1o = r1 >= cfg.q1
        ovf1[cpu] = np.bincount(t1[m1o], minlength=n_tiles)

        # L2: sort by (c, t, d128)
        o2 = np.lexsort((d128, tl, cch))
        c2, t2, dd2, i2 = cch[o2], tl[o2], d128[o2], idxv[o2]
        r2 = _ranks((c2 * n_tiles + t2) * P + dd2)
        m2o = r2 >= cfg.q2
        ovf2[cpu] = np.bincount((c2 * n_tiles + t2)[m2o],
                                minlength=cfg.src_chunks * n_tiles)
        cores.append((o1, t1, dd1, s1v, r1, o2, c2, t2, dd2, i2, r2, sc, dc))

    C1 = np.maximum(1, -(-ovf1.max(axis=0) // P))            # [n_tiles]
    C2 = np.maximum(1, -(-ovf2.max(axis=0) // P)).reshape(cfg.src_chunks, n_tiles)

    # ---- shared column layouts ----
    # L1: per st: for t: nS1(t) structural cols then C1(t) ovf cols
    nS1 = [4 * cfg.slices_of_tile(t) for t in range(n_tiles)]
    col1_struct = np.zeros(n_tiles, dtype=np.int64)   # base of structural cols
    col1_ovf = np.zeros(n_tiles, dtype=np.int64)
    tile_chunks1 = [[] for _ in range(n_tiles)]       # (col, kind, j, ovfidx)
    l1_st_cols = []
    ovfidx1 = np.zeros(n_tiles, dtype=np.int64)
    nc1 = 0
    nov1 = 0
    for st in range(n_st):
        st_start = nc1
        for t in cfg.tiles_of_st(st):
            col1_struct[t] = nc1
            for j in range(cfg.slices_of_tile(t)):
                for k in range(4):
                    tile_chunks1[t].append((nc1, 's', j, -1))
                    nc1 += 1
            col1_ovf[t] = nc1
            ovfidx1[t] = nov1
            for q in range(int(C1[t])):
                tile_chunks1[t].append((nc1, 'o', -1, nov1))
                nc1 += 1
                nov1 += 1
        l1_st_cols.append((st_start, nc1))
    NC1, NOV1 = nc1, nov1

    # L2: per (st, c): for t: q2//S structural cols + C2(c,t) ovf cols
    col2_struct = np.zeros((cfg.src_chunks, n_tiles), dtype=np.int64)
    col2_ovf = np.zeros((cfg.src_chunks, n_tiles), dtype=np.int64)
    ovfidx2 = np.zeros((cfg.src_chunks, n_tiles), dtype=np.int64)
    tile_chunks2 = {}                                 # (c,t) -> list
    st_meta2 = [[None] * cfg.src_chunks for _ in range(n_st)]
    nc2 = 0
    nov2 = 0
    for st in range(n_st):
        for c in range(cfg.src_chunks):
            colstart = nc2
            for t in cfg.tiles_of_st(st):
                lst = []
                col2_struct[c, t] = nc2
                for j in range(cfg.slices_of_tile(t)):
                    lst.append((nc2, 's', j, -1))
                    nc2 += 1
                col2_ovf[c, t] = nc2
                ovfidx2[c, t] = nov2
                for q in range(int(C2[c, t])):
                    lst.append((nc2, 'o', -1, nov2))
                    nc2 += 1
                    nov2 += 1
                tile_chunks2[(c, t)] = lst
            G = nc2 - colstart
            st_meta2[st][c] = (colstart, G, G * P)
    NC2, NOV2 = nc2, nov2

    # ---- pass 2: per-core device arrays ----
    dev = []
    for cpu in range(cfg.n_cores):
        (o1, t1, dd1, s1v, r1, o2, c2, t2, dd2, i2, r2, sc, dc) = cores[cpu]

        # L1 stream positions
        j1 = dd1 // 16
        p16_1 = dd1 % 16
        ms = r1 < cfg.q1
        col_s = col1_struct[t1[ms]] + j1[ms] * 4 + r1[ms] // S
        row_s = p16_1[ms] * S + r1[ms] % S
        pos_s = col_s * P + row_s
        mo = ~ms
        to = t1[mo]
        orank = _ranks(to)  # overflow edges sorted by t already
        col_o = col1_ovf[to] + orank // P
        row_o = orank % P
        pos_o = col_o * P + row_o

        xs1 = np.zeros((NC1 * P, F_IN), dtype=np.float16)
        xs1[pos_s] = xsrc_all[s1v[ms]]
        xs1[pos_o] = xsrc_all[s1v[mo]]
        dv1 = np.full(NOV1 * P, -1.0, dtype=np.float32)
        dv1[(ovfidx1[to] + orank // P) * P + row_o] = dd1[mo].astype(np.float32)

        xs = np.ascontiguousarray(xs1.reshape(NC1, P, F_IN).transpose(1, 0, 2))
        dv1w = np.ascontiguousarray(dv1.reshape(NOV1, P).T)

        # L2 stream positions
        j2 = dd2 // 16
        p16_2 = dd2 % 16
        ms2 = r2 < cfg.q2
        colb = col2_struct[c2[ms2], t2[ms2]] + j2[ms2]
        rowb = p16_2[ms2] * S + r2[ms2]
        pos2s = colb * P + rowb
        mo2 = ~ms2
        key_o2 = c2[mo2] * n_tiles + t2[mo2]
        orank2 = _ranks(key_o2)
        col_o2 = col2_ovf[c2[mo2], t2[mo2]] + orank2 // P
        row_o2 = orank2 % P
        pos2o = col_o2 * P + row_o2

        idx2 = np.zeros(NC2 * P, dtype=np.int16)
        idx2[pos2s] = i2[ms2].astype(np.int16)
        idx2[pos2o] = i2[mo2].astype(np.int16)
        dv2 = np.full(NOV2 * P, -1.0, dtype=np.float32)
        dv2[(ovfidx2[c2[mo2], t2[mo2]] + orank2 // P) * P + row_o2] = \
            dd2[mo2].astype(np.float32)

        idx_w = np.tile(idx2.reshape(NC2 * 8, 16).T, (8, 1))
        dv2w = np.ascontiguousarray(dv2.reshape(NOV2, P).T)

        s_core = np.zeros(n_tiles * P, dtype=np.float32)
        s_core[:cfg.shard] = s[cpu * cfg.shard:(cpu + 1) * cfg.shard]
        s_nm = s_core.reshape(n_tiles, P).T.copy()

        dev.append(dict(xs=xs, dv1=dv1w, idx=np.ascontiguousarray(idx_w),
                        dv2=dv2w, s_nm=s_nm))

    # structural selection matrices: Mst[r, j, c] = 1 if c == 16j + r//8
    Mst = np.zeros((P, 8, P), dtype=np.float16)
    r = np.arange(P)
    for j in range(8):
        Mst[r, j, 16 * j + r // S] = 1.0

    wb = dict(
        W1=np.asarray(W1, np.float32), W2=np.asarray(W2, np.float32),
        Wf=np.asarray(Wf, np.float32),
        B1=np.broadcast_to(np.asarray(b1, np.float32).reshape(1, cfg.f_hid),
                           (P, cfg.f_hid)).copy(),
        b2=np.asarray(b2, np.float32).reshape(cfg.f_hid, 1),
        bf=np.asarray(bf, np.float32).reshape(cfg.f_out, 1),
        Mst=Mst,
    )
    sched = dict(NC1=NC1, NOV1=NOV1, NC2=NC2, NOV2=NOV2,
                 l1_st_cols=l1_st_cols, tile_chunks1=tile_chunks1,
                 st_meta2=st_meta2, tile_chunks2=tile_chunks2)
    return dev, wb, sched


def build(cfg, sched):
    import concourse.bass as bass
    import concourse.mybir as mybir
    import concourse.tile as tile
    from concourse import bacc

    dt = mybir.dt
    F_IN, F_HID, F_OUT = cfg.f_in, cfg.f_hid, cfg.f_out
    NC1, NOV1 = sched["NC1"], sched["NOV1"]
    NC2, NOV2 = sched["NC2"], sched["NOV2"]
    l1_st_cols = sched["l1_st_cols"]
    tile_chunks1 = sched["tile_chunks1"]
    st_meta2 = sched["st_meta2"]
    tile_chunks2 = sched["tile_chunks2"]

    nc = bacc.Bacc("TRN2", target_bir_lowering=False, num_devices=cfg.n_cores)
    xs_in = nc.declare_dram_parameter("xs", [P, NC1, F_IN], dt.float16, isOutput=False)
    dv1_in = nc.declare_dram_parameter("dv1", [P, NOV1], dt.float32, isOutput=False)
    idx_in = nc.declare_dram_parameter("idx", [P, NC2 * 8], dt.int16, isOutput=False)
    dv2_in = nc.declare_dram_parameter("dv2", [P, NOV2], dt.float32, isOutput=False)
    snm_in = nc.declare_dram_parameter("s_nm", [P, cfg.n_tiles], dt.float32, isOutput=False)
    Mst_in = nc.declare_dram_parameter("Mst", [P, 8, P], dt.float16, isOutput=False)
    W1_in = nc.declare_dram_parameter("W1", [F_IN, F_HID], dt.float32, isOutput=False)
    W2_in = nc.declare_dram_parameter("W2", [F_HID, F_HID], dt.float32, isOutput=False)
    Wf_in = nc.declare_dram_parameter("Wf", [F_HID, F_OUT], dt.float32, isOutput=False)
    B1_in = nc.declare_dram_parameter("B1", [P, F_HID], dt.float32, isOutput=False)
    b2_in = nc.declare_dram_parameter("b2", [F_HID, 1], dt.float32, isOutput=False)
    bf_in = nc.declare_dram_parameter("bf", [F_OUT, 1], dt.float32, isOutput=False)
    out_ext = nc.declare_dram_parameter("out_fm", [F_OUT, cfg.shard], dt.float32, isOutput=True)

    def thin_gather(out_ap, in_ap, idxs_ap, num_idxs):
        """dma_gather fetching the first 128B of each 256B-strided table row."""
        eng = nc.gpsimd
        _in_ap = eng.lower_ap_dma(in_ap, for_custom_bir_dma=True)
        _idxs_ap = eng.lower_ap(idxs_ap)
        _out_ap = eng.lower_ap(out_ap)
        return eng.add_instruction(
            mybir.InstDMAGatherAnt(
                name=eng.bass.get_next_instruction_name(),
                ins=[*_in_ap, _idxs_ap, eng.lower_val_access(eng.to_reg(num_idxs))],
                outs=[_out_ap],
                transpose=False,
                num_idxs=num_idxs,
                elem_size=F_HID,
                stride_bytes_256=1,
                gen_mode=0,
                single_packet=False,
                queue_num=0,
                sbuf_tokens_per_rank=0,
                sbuf_free_dim_per_rank=0,
                sbuf_free_dim_pad_per_rank=0,
                sbuf_byte_offset=0,
            )
        )

    with tile.TileContext(nc, num_cores=cfg.n_cores) as tc, ExitStack() as ctx:
        dram = ctx.enter_context(tc.tile_pool(name="dram", bufs=1, space="DRAM"))
        const = ctx.enter_context(tc.tile_pool(name="const", bufs=1))
        mpool = ctx.enter_context(tc.tile_pool(name="mpool", bufs=12))
        evac = ctx.enter_context(tc.tile_pool(name="evac", bufs=6))

        h1loc = dram.tile([cfg.shard, F_HID], dt.float16)
        # gather tables: 2 halves x [2 chunks, 1 zero row + 25000 rows, FP]
        h1tab0 = dram.tile([2, CJ + 1, FP], dt.float16, name="h1tab0")
        h1tab1 = dram.tile([2, CJ + 1, FP], dt.float16, name="h1tab1")
        # compact AllGather landing buffers (collective outs must be contiguous)
        h1cmp0 = dram.tile([2, CJ, F_HID], dt.float16, name="h1cmp0")
        h1cmp1 = dram.tile([2, CJ, F_HID], dt.float16, name="h1cmp1")

        iota_i = const.tile([P, P], dt.int16)
        nc.gpsimd.iota(iota_i[:], pattern=[[1, P]], base=0, channel_multiplier=0)
        iota16 = const.tile([P, P], dt.float16)
        nc.vector.tensor_copy(iota16[:], iota_i[:])
        iotapP = const.tile([P, 1], dt.int16)
        nc.gpsimd.iota(iotapP[:], pattern=[[0, 1]], base=0, channel_multiplier=1)
        iotapPf = const.tile([P, 1], dt.float32)
        nc.vector.tensor_copy(iotapPf[:], iotapP[:])
        identP = const.tile([P, P], dt.float16)
        nc.vector.tensor_scalar(out=identP[:], in0=iota16[:], scalar1=iotapPf[:, 0:1],
                                scalar2=None, op0=mybir.AluOpType.is_equal)
        zrow = const.tile([2, FP], dt.float16)
        nc.vector.memset(zrow[:], 0.0)
        for half in range(2):
            nc.sync.dma_start(h1tab0[half, 0:1, :], zrow[half:half + 1, :])
            nc.sync.dma_start(h1tab1[half, 0:1, :], zrow[half:half + 1, :])

        W1s = const.tile([F_IN, F_HID], dt.float32)
        W2s = const.tile([F_HID, F_HID], dt.float32)
        Wfs = const.tile([F_HID, F_OUT], dt.float32)
        B1s = const.tile([P, F_HID], dt.float32)
        b2s = const.tile([F_HID, 1], dt.float32)
        bfs = const.tile([F_OUT, 1], dt.float32)
        snm = const.tile([P, cfg.n_tiles], dt.float32)
        Msts = const.tile([P, 8, P], dt.float16)
        dv1s = const.tile([P, NOV1], dt.float32)
        dv2s = const.tile([P, NOV2], dt.float32)
        nc.sync.dma_start(W1s[:], W1_in[:])
        nc.sync.dma_start(W2s[:], W2_in[:])
        nc.sync.dma_start(Wfs[:], Wf_in[:])
        nc.sync.dma_start(B1s[:], B1_in[:])
        nc.sync.dma_start(b2s[:], b2_in[:])
        nc.sync.dma_start(bfs[:], bf_in[:])
        nc.sync.dma_start(snm[:], snm_in[:])
        nc.sync.dma_start(Msts[:], Mst_in[:])
        nc.sync.dma_start(dv1s[:], dv1_in[:])
        nc.sync.dma_start(dv2s[:], dv2_in[:])

        W1s16 = const.tile([F_IN, F_HID], dt.float16)
        nc.scalar.activation(W1s16[:], W1s[:], mybir.ActivationFunctionType.Copy)
        W2s16 = const.tile([F_HID, F_HID], dt.float16)
        nc.scalar.activation(W2s16[:], W2s[:], mybir.ActivationFunctionType.Copy)
        Wfs16 = const.tile([F_HID, F_OUT], dt.float16)
        nc.scalar.activation(Wfs16[:], Wfs[:], mybir.ActivationFunctionType.Copy)

        def build_M(scol):
            """Indicator matrix [128e, 128d] = (iota == dstv) on DVE."""
            M16 = mpool.tile([P, P], dt.float16, tag="M")
            nc.vector.tensor_scalar(
                out=M16[:], in0=iota16[:], scalar1=scol, scalar2=None,
                op0=mybir.AluOpType.is_equal)
            return M16

        # ---------------- layer 1 ----------------
        hs2 = cfg.shard // 2
        ag_after = {}
        for half, (a, b), tab, cmp_ in ((0, (0, hs2), h1tab0, h1cmp0),
                                        (1, (hs2, 2 * hs2), h1tab1, h1cmp1)):
            last_tile = (b - 1) // P
            st_of = last_tile // cfg.st_tiles
            ag_after[st_of] = ((a, b), tab, cmp_)

        expands = []

        def emit_ag(st):
            if st not in ag_after:
                return
            (a, b), tab, cmp_ = ag_after.pop(st)
            nc.gpsimd.collective_compute(
                "AllGather", mybir.AluOpType.bypass,
                replica_groups=[list(range(cfg.n_cores))],
                ins=[h1loc[a:b, :].opt()],
                outs=[cmp_[:].opt()],
            )
            # expand into the 256B-strided gather table is deferred: emitted on
            # the Act HWDGE queue at a point where its sem wait cannot block
            # queued work that must run before the collective completes.
            expands.append((tab, cmp_))

        def emit_expand(i):
            tab, cmp_ = expands[i]
            nc.scalar.dma_start(tab[:, 1:, 0:F_HID], cmp_[:])

        with tc.tile_pool(name="l1s", bufs=3) as l1s, \
             tc.tile_pool(name="l1p", bufs=2, space="PSUM") as l1p:
            for st in range(cfg.n_st):
                colstart, colend = l1_st_cols[st]
                n_stc = colend - colstart
                xs_st = l1s.tile([P, n_stc, F_IN], dt.float16, tag="xs")
                nc.sync.dma_start(xs_st[:], xs_in[:, colstart:colend, :])

                for t in cfg.tiles_of_st(st):
                    chunks = tile_chunks1[t]
                    acc1 = l1p.tile([P, F_IN], dt.float32, tag="acc1", bufs=3)
                    for ci, (col, kind, j, oidx) in enumerate(chunks):
                        lhs = Msts[:, j, :] if kind == 's' else \
                            build_M(dv1s[:, oidx:oidx + 1])[:]
                        nc.tensor.matmul(
                            acc1[:], lhsT=lhs, rhs=xs_st[:, col - colstart, :],
                            start=(ci == 0), stop=(ci == len(chunks) - 1))
                    # dense: scale by s_dst -> transpose -> W1 -> +b1, relu, *s
                    a1n = evac.tile([P, F_IN], dt.float16, tag="a1n")
                    nc.vector.tensor_scalar(out=a1n[:], in0=acc1[:],
                                            scalar1=snm[:, t:t + 1], scalar2=None,
                                            op0=mybir.AluOpType.mult)
                    a1T = l1p.tile([F_IN, P], dt.float16, tag="a1T", bufs=1)
                    nc.tensor.transpose(a1T[:], a1n[:], identP[:])
                    a1s = evac.tile([F_IN, P], dt.float16, tag="a1s")
                    nc.scalar.activation(a1s[:], a1T[:], mybir.ActivationFunctionType.Copy)
                    z1p = l1p.tile([P, F_HID], dt.float32, tag="z1p", bufs=2)
                    nc.tensor.matmul(z1p[:], lhsT=a1s[:], rhs=W1s16[:], start=True, stop=True)
                    t1 = evac.tile([P, F_HID], dt.float16, tag="t1")
                    nc.vector.tensor_tensor(out=t1[:], in0=z1p[:], in1=B1s[:],
                                            op=mybir.AluOpType.add)
                    h1k = evac.tile([P, F_HID], dt.float16, tag="h1k")
                    nc.vector.tensor_scalar(
                        out=h1k[:], in0=t1[:], scalar1=snm[:, t:t + 1],
                        scalar2=0.0, op0=mybir.AluOpType.mult, op1=mybir.AluOpType.max)
                    rows = cfg.rows_of_tile(t)
                    nc.sync.dma_start(h1loc[t * P:t * P + rows, :], h1k[:rows, :])
                emit_ag(st)

        # ---------------- layer 2 ----------------
        with tc.tile_pool(name="l2s", bufs=4) as l2s, \
             tc.tile_pool(name="gpool", bufs=5) as gpool, \
             tc.tile_pool(name="accp", bufs=cfg.n_st) as accp, \
             tc.tile_pool(name="l2p", bufs=1, space="PSUM") as l2p:
            accs = {}
            srcs = {0: h1tab0[0], 1: h1tab0[1], 2: h1tab1[0], 3: h1tab1[1]}

            def emit_chunk_group(st, c):
                tiles = cfg.tiles_of_st(st)
                if c == 0:
                    acc_st = accp.tile([P, len(tiles), F_HID], dt.float16,
                                       tag="accS", name=f"accS{st}")
                    accs[st] = acc_st
                colstart, G, nidx = st_meta2[st][c]
                idx_t = l2s.tile([P, G * 8], dt.int16, tag="idx")
                nc.sync.dma_start(idx_t[:], idx_in[:, colstart * 8:(colstart + G) * 8])
                gt = gpool.tile([P, G, F_HID], dt.float16, tag="gath")
                thin_gather(gt[:], srcs[c][:, 0:F_HID], idx_t[:], nidx)
                for ti, t in enumerate(tiles):
                    chs = tile_chunks2[(c, t)]
                    acc = l2p.tile([P, F_HID], dt.float32, tag="accq", bufs=4)
                    for ci, (col, kind, j, oidx) in enumerate(chs):
                        lhs = Msts[:, j, :] if kind == 's' else \
                            build_M(dv2s[:, oidx:oidx + 1])[:]
                        nc.tensor.matmul(
                            acc[:], lhsT=lhs, rhs=gt[:, col - colstart, :],
                            start=(ci == 0), stop=(ci == len(chs) - 1))
                    sl = accs[st][:, ti, :]
                    if c == 0:
                        nc.scalar.activation(sl, acc[:], mybir.ActivationFunctionType.Copy)
                    else:
                        nc.vector.tensor_add(sl, acc[:], sl)

            def emit_final(st):
                tiles = cfg.tiles_of_st(st)
                for ti, t in enumerate(tiles):
                    sl = accs[st][:, ti, :]
                    slf = evac.tile([P, F_HID], dt.float16, tag="slf")
                    nc.vector.tensor_scalar(out=slf[:], in0=sl, scalar1=snm[:, t:t + 1],
                                            scalar2=None, op0=mybir.AluOpType.mult)
                    aT = l2p.tile([F_HID, P], dt.float16, tag="aT", bufs=1)
                    nc.tensor.transpose(aT[:], slf[:], identP[:])
                    a2s = evac.tile([F_HID, P], dt.float16, tag="a2s")
                    nc.scalar.activation(a2s[:], aT[:], mybir.ActivationFunctionType.Copy)
                    ph2 = l2p.tile([F_HID, P], dt.float32, tag="ph2", bufs=2)
                    nc.tensor.matmul(ph2[:], lhsT=W2s16[:], rhs=a2s[:], start=True, stop=True)
                    h2f = evac.tile([F_HID, P], dt.float16, tag="h2f")
                    nc.scalar.activation(h2f[:], ph2[:], mybir.ActivationFunctionType.Relu,
                                         bias=b2s[:, 0:1])
                    po = l2p.tile([F_OUT, P], dt.float32, tag="po", bufs=1)
                    nc.tensor.matmul(po[:], lhsT=Wfs16[:], rhs=h2f[:], start=True, stop=True)
                    osb = evac.tile([F_OUT, P], dt.float32, tag="osb")
                    nc.scalar.activation(osb[:], po[:], mybir.ActivationFunctionType.Identity,
                                         bias=bfs[:, 0:1])
                    rows = cfg.rows_of_tile(t)
                    nc.sync.dma_start(out_ext[:, t * P:t * P + rows], osb[:, :rows])

            emit_expand(0)
            for st in range(cfg.n_st):
                emit_chunk_group(st, 0)
                emit_chunk_group(st, 1)
            emit_expand(1)
            for st in range(cfg.n_st):
                emit_chunk_group(st, 2)
                emit_chunk_group(st, 3)
                emit_final(st)

    nc.finalize()
    return nc


def make_in_maps(cfg, dev, wb):
    maps = []
    for cpu in range(cfg.n_cores):
        d = dev[cpu]
        maps.append({
            "xs": d["xs"], "dv1": d["dv1"], "idx": d["idx"], "dv2": d["dv2"],
            "s_nm": d["s_nm"],
            **{k: wb[k] for k in ("W1", "W2", "Wf", "B1", "b2", "bf", "Mst")},
        })
    return maps


def kernel(x, edge_index, W1, b1, W2, b2, Wf, bf, _trace=False, _tmpdir=None):
    from concourse.bass_utils import run_bass_kernel_spmd

    cfg = CFG
    dev, wb, sched = preprocess(cfg, x, edge_index, W1, b1, W2, b2, Wf, bf)
    nc = build(cfg, sched)
    in_maps = make_in_maps(cfg, dev, wb)
    res = run_bass_kernel_spmd(nc, in_maps, core_ids=list(range(cfg.n_cores)),
                               trace=_trace, tmpdir=_tmpdir)
    out = np.concatenate([res.results[c]["out_fm"].T for c in range(cfg.n_cores)], axis=0)
    kernel._last_results = res
    return out.astype(np.float32)


# revision 9
# speedup vs baseline: 1.3069x; 1.0209x over previous
"""Trainium2 Bass kernel for a 2-layer GCN (GCNConv -> relu -> GCNConv -> relu -> Linear).

Math: with s = deg^-1/2 (deg over dst incl. self-loops):
  h1 = relu( s_d * (A_ind @ (s_s * x)) @ W1 + b1 )   (aggregate 4-wide first)
  h2 = relu( W2 @ (s_d * (A_ind @ h1')) + b2 ),  h1' = h1 * s  (table prescale)
  out = h2 @ Wf + bf
A_ind is the pure 0/1 edge indicator: s_src is folded into the streamed
features (host), s_dst applied per dst-tile after aggregation.

Device strategy (8 cores, nodes sharded by dst, SPMD one program):
  - STRUCTURAL SLOTS: each dst gets a fixed quota of edge slots (L1: 32 per
    tile; L2: 8 per (dst, src_chunk)); a 128-slot chunk covers 16 consecutive
    dst x 8 slots, aggregated with a CONSTANT block-diagonal selection matrix
    Mst[:, j, :] (one per 16-dst slice, host-uploaded) -- no per-chunk M build.
    Overflow edges beyond the quota go through classic built-M chunks
    (DVE is_equal vs iota), but those are now rare (~15% of chunks).
  - slot padding: L1 pad slots stream zero features; L2 pad slots gather a
    reserved zero row (row 0 of each 25001-row table chunk, idx 0; real rows
    at idx 1+trow%25000).
  - AllGather moves only the 64 real fp16 feature columns (strided out AP into
    the 256B-row padded gather table) in 2 halves emitted inside the L1 loop.
  - layer-2 source rows fetched per-slot via raw InstDMAGatherAnt (elem_size
    64, 256B stride -> 128B payload/descriptor).
"""
import numpy as np
from contextlib import ExitStack
from dataclasses import dataclass


@dataclass(frozen=True)
class Cfg:
    n_nodes: int = 100000
    n_cores: int = 8
    f_in: int = 4
    f_hid: int = 64
    f_out: int = 2
    src_chunks: int = 4
    st_tiles: int = 4
    q1: int = 32   # L1 structural slots per dst (4 chunks x 8)
    q2: int = 8    # L2 structural slots per (dst, src_chunk) (1 chunk x 8)

    @property
    def shard(self):
        return self.n_nodes // self.n_cores

    @property
    def n_tiles(self):
        return (self.shard + 127) // 128

    @property
    def last_rows(self):
        return self.shard - (self.n_tiles - 1) * 128

    @property
    def n_st(self):
        return (self.n_tiles + self.st_tiles - 1) // self.st_tiles

    def tiles_of_st(self, st):
        return list(range(st * self.st_tiles, min((st + 1) * self.st_tiles, self.n_tiles)))

    def rows_of_tile(self, t):
        return self.last_rows if t == self.n_tiles - 1 else 128

    def slices_of_tile(self, t):
        return (self.rows_of_tile(t) + 15) // 16


CFG = Cfg()
P = 128
FP = 128   # fp16 table row padded to 256B (dma_gather min stride)
S = 8      # slots per dst per structural chunk
HS = CFG.shard // 2          # rows per AllGather half per core
CJ = HS * CFG.n_cores // 2   # 25000 rows per table idx chunk


def _ranks(sorted_keys):
    """rank of each element within its run of equal (sorted) keys."""
    n = len(sorted_keys)
    if n == 0:
        return np.zeros(0, dtype=np.int64)
    first = np.empty(n, dtype=bool)
    first[0] = True
    np.not_equal(sorted_keys[1:], sorted_keys[:-1], out=first[1:])
    gstart = np.flatnonzero(first)
    gid = np.cumsum(first) - 1
    return np.arange(n) - gstart[gid]


def preprocess(cfg, x, edge_index, W1, b1, W2, b2, Wf, bf):
    """Host-side sharding: slot assignment, overflow grouping, streams."""
    F_IN = cfg.f_in
    src0 = np.asarray(edge_index[0], dtype=np.int64)
    dst0 = np.asarray(edge_index[1], dtype=np.int64)
    deg = (np.bincount(dst0, minlength=cfg.n_nodes) + 1).astype(np.float64)
    s = (1.0 / np.sqrt(deg)).astype(np.float32)

    # self loops appended as ordinary edges
    loop = np.arange(cfg.n_nodes, dtype=np.int64)
    src = np.concatenate([src0, loop])
    dst = np.concatenate([dst0, loop])
    x = np.asarray(x, dtype=np.float32)
    xsrc_all = (x * s[:, None]).astype(np.float16)  # prescaled by s_src

    core_id = dst // cfg.shard

    # L2 table mapping for each global src node
    lr_all = src % cfg.shard
    score_all = src // cfg.shard
    h_all = lr_all // HS
    trow_all = score_all * HS + (lr_all - h_all * HS)
    c_all = h_all * 2 + trow_all // CJ          # src chunk 0..3
    idx_all = 1 + (trow_all % CJ)               # 1-based; 0 = zero row

    n_tiles, n_st = cfg.n_tiles, cfg.n_st

    # ---- pass 1: per-core sorted edge views + overflow counts ----
    cores = []
    ovf1 = np.zeros((cfg.n_cores, n_tiles), dtype=np.int64)
    ovf2 = np.zeros((cfg.n_cores, cfg.src_chunks * n_tiles), dtype=np.int64)
    for cpu in range(cfg.n_cores):
        m = core_id == cpu
        sc, dc = src[m], dst[m]
        dl = dc - cpu * cfg.shard
        tl = dl // P
        d128 = dl % P
        cch = c_all[m]
        idxv = idx_all[m]

        # L1: sort by (t, d128)
        o1 = np.lexsort((d128, tl))
        t1, dd1, s1v = tl[o1], d128[o1], sc[o1]
        r1 = _ranks(t1 * P + dd1)
        m1o = r1 >= cfg.q1
        ovf1[cpu] = np.bincount(t1[m1o], minlength=n_tiles)

        # L2: sort by (c, t, d128)
        o2 = np.lexsort((d128, tl, cch))
        c2, t2, dd2, i2 = cch[o2], tl[o2], d128[o2], idxv[o2]
        r2 = _ranks((c2 * n_tiles + t2) * P + dd2)
        m2o = r2 >= cfg.q2
        ovf2[cpu] = np.bincount((c2 * n_tiles + t2)[m2o],
                                minlength=cfg.src_chunks * n_tiles)
        cores.append((o1, t1, dd1, s1v, r1, o2, c2, t2, dd2, i2, r2, sc, dc))

    C1 = np.maximum(1, -(-ovf1.max(axis=0) // P))            # [n_tiles]
    C2 = np.maximum(1, -(-ovf2.max(axis=0) // P)).reshape(cfg.src_chunks, n_tiles)

    # ---- shared column layouts ----
    # L1: per st: for t: nS1(t) structural cols then C1(t) ovf cols
    nS1 = [4 * cfg.slices_of_tile(t) for t in range(n_tiles)]
    col1_struct = np.zeros(n_tiles, dtype=np.int64)   # base of structural cols
    col1_ovf = np.zeros(n_tiles, dtype=np.int64)
    tile_chunks1 = [[] for _ in range(n_tiles)]       # (col, kind, j, ovfidx)
    l1_st_cols = []
    ovfidx1 = np.zeros(n_tiles, dtype=np.int64)
    nc1 = 0
    nov1 = 0
    for st in range(n_st):
        st_start = nc1
        for t in cfg.tiles_of_st(st):
            col1_struct[t] = nc1
            for j in range(cfg.slices_of_tile(t)):
                for k in range(4):
                    tile_chunks1[t].append((nc1, 's', j, -1))
                    nc1 += 1
            col1_ovf[t] = nc1
            ovfidx1[t] = nov1
            for q in range(int(C1[t])):
                tile_chunks1[t].append((nc1, 'o', -1, nov1))
                nc1 += 1
                nov1 += 1
        l1_st_cols.append((st_start, nc1))
    NC1, NOV1 = nc1, nov1

    # L2: per (st, c): for t: q2//S structural cols + C2(c,t) ovf cols
    col2_struct = np.zeros((cfg.src_chunks, n_tiles), dtype=np.int64)
    col2_ovf = np.zeros((cfg.src_chunks, n_tiles), dtype=np.int64)
    ovfidx2 = np.zeros((cfg.src_chunks, n_tiles), dtype=np.int64)
    tile_chunks2 = {}                                 # (c,t) -> list
    st_meta2 = [[None] * cfg.src_chunks for _ in range(n_st)]
    nc2 = 0
    nov2 = 0
    for st in range(n_st):
        for c in range(cfg.src_chunks):
            colstart = nc2
            for t in cfg.tiles_of_st(st):
                lst = []
                col2_struct[c, t] = nc2
                for j in range(cfg.slices_of_tile(t)):
                    lst.append((nc2, 's', j, -1))
                    nc2 += 1
                col2_ovf[c, t] = nc2
                ovfidx2[c, t] = nov2
                for q in range(int(C2[c, t])):
                    lst.append((nc2, 'o', -1, nov2))
                    nc2 += 1
                    nov2 += 1
                tile_chunks2[(c, t)] = lst
            G = nc2 - colstart
            st_meta2[st][c] = (colstart, G, G * P)
    NC2, NOV2 = nc2, nov2

    # ---- pass 2: per-core device arrays ----
    dev = []
    for cpu in range(cfg.n_cores):
        (o1, t1, dd1, s1v, r1, o2, c2, t2, dd2, i2, r2, sc, dc) = cores[cpu]

        # L1 stream positions
        j1 = dd1 // 16
        p16_1 = dd1 % 16
        ms = r1 < cfg.q1
        col_s = col1_struct[t1[ms]] + j1[ms] * 4 + r1[ms] // S
        row_s = p16_1[ms] * S + r1[ms] % S
        pos_s = col_s * P + row_s
        mo = ~ms
        to = t1[mo]
        orank = _ranks(to)  # overflow edges sorted by t already
        col_o = col1_ovf[to] + orank // P
        row_o = orank % P
        pos_o = col_o * P + row_o

        xs1 = np.zeros((NC1 * P, F_IN), dtype=np.float16)
        xs1[pos_s] = xsrc_all[s1v[ms]]
        xs1[pos_o] = xsrc_all[s1v[mo]]
        dv1 = np.full(NOV1 * P, -1.0, dtype=np.float32)
        dv1[(ovfidx1[to] + orank // P) * P + row_o] = dd1[mo].astype(np.float32)

        xs = np.ascontiguousarray(xs1.reshape(NC1, P, F_IN).transpose(1, 0, 2))
        dv1w = np.ascontiguousarray(dv1.reshape(NOV1, P).T)

        # L2 stream positions
        j2 = dd2 // 16
        p16_2 = dd2 % 16
        ms2 = r2 < cfg.q2
        colb = col2_struct[c2[ms2], t2[ms2]] + j2[ms2]
        rowb = p16_2[ms2] * S + r2[ms2]
        pos2s = colb * P + rowb
        mo2 = ~ms2
        key_o2 = c2[mo2] * n_tiles + t2[mo2]
        orank2 = _ranks(key_o2)
        col_o2 = col2_ovf[c2[mo2], t2[mo2]] + orank2 // P
        row_o2 = orank2 % P
        pos2o = col_o2 * P + row_o2

        idx2 = np.zeros(NC2 * P, dtype=np.int16)
        idx2[pos2s] = i2[ms2].astype(np.int16)
        idx2[pos2o] = i2[mo2].astype(np.int16)
        dv2 = np.full(NOV2 * P, -1.0, dtype=np.float32)
        dv2[(ovfidx2[c2[mo2], t2[mo2]] + orank2 // P) * P + row_o2] = \
            dd2[mo2].astype(np.float32)

        idx_w = np.tile(idx2.reshape(NC2 * 8, 16).T, (8, 1))
        dv2w = np.ascontiguousarray(dv2.reshape(NOV2, P).T)

        s_core = np.zeros(n_tiles * P, dtype=np.float32)
        s_core[:cfg.shard] = s[cpu * cfg.shard:(cpu + 1) * cfg.shard]
        s_nm = s_core.reshape(n_tiles, P).T.copy()

        dev.append(dict(xs=xs, dv1=dv1w, idx=np.ascontiguousarray(idx_w),
                        dv2=dv2w, s_nm=s_nm))

    # structural selection matrices: Mst[r, j, c] = 1 if c == 16j + r//8
    Mst = np.zeros((P, 8, P), dtype=np.float16)
    r = np.arange(P)
    for j in range(8):
        Mst[r, j, 16 * j + r // S] = 1.0

    wb = dict(
        W1=np.asarray(W1, np.float32), W2=np.asarray(W2, np.float32),
        Wf=np.asarray(Wf, np.float32),
        B1=np.broadcast_to(np.asarray(b1, np.float32).reshape(1, cfg.f_hid),
                           (P, cfg.f_hid)).copy(),
        b2=np.asarray(b2, np.float32).reshape(cfg.f_hid, 1),
        bf=np.asarray(bf, np.float32).reshape(cfg.f_out, 1),
        Mst=Mst,
    )
    sched = dict(NC1=NC1, NOV1=NOV1, NC2=NC2, NOV2=NOV2,
                 l1_st_cols=l1_st_cols, tile_chunks1=tile_chunks1,
                 st_meta2=st_meta2, tile_chunks2=tile_chunks2)
    return dev, wb, sched


def build(cfg, sched):
    import concourse.bass as bass
    import concourse.mybir as mybir
    import concourse.tile as tile
    from concourse import bacc

    dt = mybir.dt
    F_IN, F_HID, F_OUT = cfg.f_in, cfg.f_hid, cfg.f_out
    NC1, NOV1 = sched["NC1"], sched["NOV1"]
    NC2, NOV2 = sched["NC2"], sched["NOV2"]
    l1_st_cols = sched["l1_st_cols"]
    tile_chunks1 = sched["tile_chunks1"]
    st_meta2 = sched["st_meta2"]
    tile_chunks2 = sched["tile_chunks2"]

    nc = bacc.Bacc("TRN2", target_bir_lowering=False, num_devices=cfg.n_cores)
    xs_in = nc.declare_dram_parameter("xs", [P, NC1, F_IN], dt.float16, isOutput=False)
    dv1_in = nc.declare_dram_parameter("dv1", [P, NOV1], dt.float32, isOutput=False)
    idx_in = nc.declare_dram_parameter("idx", [P, NC2 * 8], dt.int16, isOutput=False)
    dv2_in = nc.declare_dram_parameter("dv2", [P, NOV2], dt.float32, isOutput=False)
    snm_in = nc.declare_dram_parameter("s_nm", [P, cfg.n_tiles], dt.float32, isOutput=False)
    Mst_in = nc.declare_dram_parameter("Mst", [P, 8, P], dt.float16, isOutput=False)
    W1_in = nc.declare_dram_parameter("W1", [F_IN, F_HID], dt.float32, isOutput=False)
    W2_in = nc.declare_dram_parameter("W2", [F_HID, F_HID], dt.float32, isOutput=False)
    Wf_in = nc.declare_dram_parameter("Wf", [F_HID, F_OUT], dt.float32, isOutput=False)
    B1_in = nc.declare_dram_parameter("B1", [P, F_HID], dt.float32, isOutput=False)
    b2_in = nc.declare_dram_parameter("b2", [F_HID, 1], dt.float32, isOutput=False)
    bf_in = nc.declare_dram_parameter("bf", [F_OUT, 1], dt.float32, isOutput=False)
    out_ext = nc.declare_dram_parameter("out_fm", [F_OUT, cfg.shard], dt.float32, isOutput=True)

    def thin_gather(out_ap, in_ap, idxs_ap, num_idxs):
        """dma_gather fetching the first 128B of each 256B-strided table row."""
        eng = nc.gpsimd
        _in_ap = eng.lower_ap_dma(in_ap, for_custom_bir_dma=True)
        _idxs_ap = eng.lower_ap(idxs_ap)
        _out_ap = eng.lower_ap(out_ap)
        return eng.add_instruction(
            mybir.InstDMAGatherAnt(
                name=eng.bass.get_next_instruction_name(),
                ins=[*_in_ap, _idxs_ap, eng.lower_val_access(eng.to_reg(num_idxs))],
                outs=[_out_ap],
                transpose=False,
                num_idxs=num_idxs,
                elem_size=F_HID,
                stride_bytes_256=1,
                gen_mode=0,
                single_packet=False,
                queue_num=0,
                sbuf_tokens_per_rank=0,
                sbuf_free_dim_per_rank=0,
                sbuf_free_dim_pad_per_rank=0,
                sbuf_byte_offset=0,
            )
        )

    with tile.TileContext(nc, num_cores=cfg.n_cores) as tc, ExitStack() as ctx:
        dram = ctx.enter_context(tc.tile_pool(name="dram", bufs=1, space="DRAM"))
        const = ctx.enter_context(tc.tile_pool(name="const", bufs=1))
        mpool = ctx.enter_context(tc.tile_pool(name="mpool", bufs=12))
        evac = ctx.enter_context(tc.tile_pool(name="evac", bufs=6))

        h1loc = dram.tile([cfg.shard, F_HID], dt.float16)
        # gather tables: 2 halves x [2 chunks, 1 zero row + 25000 rows, FP]
        h1tab0 = dram.tile([2, CJ + 1, FP], dt.float16, name="h1tab0")
        h1tab1 = dram.tile([2, CJ + 1, FP], dt.float16, name="h1tab1")
        # compact AllGather landing buffers (collective outs must be contiguous)
        h1cmp0 = dram.tile([2, CJ, F_HID], dt.float16, name="h1cmp0")
        h1cmp1 = dram.tile([2, CJ, F_HID], dt.float16, name="h1cmp1")

        iota_i = const.tile([P, P], dt.int16)
        nc.gpsimd.iota(iota_i[:], pattern=[[1, P]], base=0, channel_multiplier=0)
        iota16 = const.tile([P, P], dt.float16)
        nc.vector.tensor_copy(iota16[:], iota_i[:])
        iotapP = const.tile([P, 1], dt.int16)
        nc.gpsimd.iota(iotapP[:], pattern=[[0, 1]], base=0, channel_multiplier=1)
        iotapPf = const.tile([P, 1], dt.float32)
        nc.vector.tensor_copy(iotapPf[:], iotapP[:])
        identP = const.tile([P, P], dt.float16)
        nc.vector.tensor_scalar(out=identP[:], in0=iota16[:], scalar1=iotapPf[:, 0:1],
                                scalar2=None, op0=mybir.AluOpType.is_equal)
        zrow = const.tile([2, FP], dt.float16)
        nc.vector.memset(zrow[:], 0.0)
        for half in range(2):
            nc.sync.dma_start(h1tab0[half, 0:1, :], zrow[half:half + 1, :])
            nc.sync.dma_start(h1tab1[half, 0:1, :], zrow[half:half + 1, :])

        W1s = const.tile([F_IN, F_HID], dt.float32)
        W2s = const.tile([F_HID, F_HID], dt.float32)
        Wfs = const.tile([F_HID, F_OUT], dt.float32)
        B1s = const.tile([P, F_HID], dt.float32)
        b2s = const.tile([F_HID, 1], dt.float32)
        bfs = const.tile([F_OUT, 1], dt.float32)
        snm = const.tile([P, cfg.n_tiles], dt.float32)
        Msts = const.tile([P, 8, P], dt.float16)
        dv1s = const.tile([P, NOV1], dt.float32)
        dv2s = const.tile([P, NOV2], dt.float32)
        nc.sync.dma_start(W1s[:], W1_in[:])
        nc.sync.dma_start(W2s[:], W2_in[:])
        nc.sync.dma_start(Wfs[:], Wf_in[:])
        nc.sync.dma_start(B1s[:], B1_in[:])
        nc.sync.dma_start(b2s[:], b2_in[:])
        nc.sync.dma_start(bfs[:], bf_in[:])
        nc.sync.dma_start(snm[:], snm_in[:])
        nc.sync.dma_start(Msts[:], Mst_in[:])
        nc.sync.dma_start(dv1s[:], dv1_in[:])
        nc.sync.dma_start(dv2s[:], dv2_in[:])

        W1s16 = const.tile([F_IN, F_HID], dt.float16)
        nc.scalar.activation(W1s16[:], W1s[:], mybir.ActivationFunctionType.Copy)
        W2s16 = const.tile([F_HID, F_HID], dt.float16)
        nc.scalar.activation(W2s16[:], W2s[:], mybir.ActivationFunctionType.Copy)
        Wfs16 = const.tile([F_HID, F_OUT], dt.float16)
        nc.scalar.activation(Wfs16[:], Wfs[:], mybir.ActivationFunctionType.Copy)

        def build_M(scol):
            """Indicator matrix [128e, 128d] = (iota == dstv) on DVE."""
            M16 = mpool.tile([P, P], dt.float16, tag="M")
            nc.vector.tensor_scalar(
                out=M16[:], in0=iota16[:], scalar1=scol, scalar2=None,
                op0=mybir.AluOpType.is_equal)
            return M16

        # ---------------- layer 1 ----------------
        hs2 = cfg.shard // 2
        ag_after = {}
        for half, (a, b), tab, cmp_ in ((0, (0, hs2), h1tab0, h1cmp0),
                                        (1, (hs2, 2 * hs2), h1tab1, h1cmp1)):
            last_tile = (b - 1) // P
            st_of = last_tile // cfg.st_tiles
            ag_after[st_of] = ((a, b), tab, cmp_)

        expands = []

        def emit_ag(st):
            if st not in ag_after:
                return
            (a, b), tab, cmp_ = ag_after.pop(st)
            nc.gpsimd.collective_compute(
                "AllGather", mybir.AluOpType.bypass,
                replica_groups=[list(range(cfg.n_cores))],
                ins=[h1loc[a:b, :].opt()],
                outs=[cmp_[:].opt()],
            )
            # expand into the 256B-strided gather table is deferred: emitted on
            # the Act HWDGE queue at a point where its sem wait cannot block
            # queued work that must run before the collective completes.
            expands.append((tab, cmp_))

        def emit_expand(i):
            tab, cmp_ = expands[i]
            nc.scalar.dma_start(tab[:, 1:, 0:F_HID], cmp_[:])

        with tc.tile_pool(name="l1s", bufs=3) as l1s, \
             tc.tile_pool(name="l1p", bufs=2, space="PSUM") as l1p:
            for st in range(cfg.n_st):
                colstart, colend = l1_st_cols[st]
                n_stc = colend - colstart
                xs_st = l1s.tile([P, n_stc, F_IN], dt.float16, tag="xs")
                nc.sync.dma_start(xs_st[:], xs_in[:, colstart:colend, :])

                for t in cfg.tiles_of_st(st):
                    chunks = tile_chunks1[t]
                    acc1 = l1p.tile([P, F_IN], dt.float32, tag="acc1", bufs=3)
                    for ci, (col, kind, j, oidx) in enumerate(chunks):
                        lhs = Msts[:, j, :] if kind == 's' else \
                            build_M(dv1s[:, oidx:oidx + 1])[:]
                        nc.tensor.matmul(
                            acc1[:], lhsT=lhs, rhs=xs_st[:, col - colstart, :],
                            start=(ci == 0), stop=(ci == len(chunks) - 1))
                    # dense: scale by s_dst -> transpose -> W1 -> +b1, relu, *s
                    a1n = evac.tile([P, F_IN], dt.float16, tag="a1n")
                    nc.vector.tensor_scalar(out=a1n[:], in0=acc1[:],
                                            scalar1=snm[:, t:t + 1], scalar2=None,
                                            op0=mybir.AluOpType.mult)
                    a1T = l1p.tile([F_IN, P], dt.float16, tag="a1T", bufs=1)
                    nc.tensor.transpose(a1T[:], a1n[:], identP[:])
                    a1s = evac.tile([F_IN, P], dt.float16, tag="a1s")
                    nc.scalar.activation(a1s[:], a1T[:], mybir.ActivationFunctionType.Copy)
                    z1p = l1p.tile([P, F_HID], dt.float32, tag="z1p", bufs=2)
                    nc.tensor.matmul(z1p[:], lhsT=a1s[:], rhs=W1s16[:], start=True, stop=True)
                    t1 = evac.tile([P, F_HID], dt.float16, tag="t1")
                    nc.vector.tensor_tensor(out=t1[:], in0=z1p[:], in1=B1s[:],
                                            op=mybir.AluOpType.add)
                    h1k = evac.tile([P, F_HID], dt.float16, tag="h1k")
                    nc.vector.tensor_scalar(
                        out=h1k[:], in0=t1[:], scalar1=snm[:, t:t + 1],
                        scalar2=0.0, op0=mybir.AluOpType.mult, op1=mybir.AluOpType.max)
                    rows = cfg.rows_of_tile(t)
                    nc.sync.dma_start(h1loc[t * P:t * P + rows, :], h1k[:rows, :])
                emit_ag(st)

        # ---------------- layer 2 ----------------
        with tc.tile_pool(name="l2s", bufs=4) as l2s, \
             tc.tile_pool(name="gpool", bufs=5) as gpool, \
             tc.tile_pool(name="accp", bufs=cfg.n_st) as accp, \
             tc.tile_pool(name="l2p", bufs=1, space="PSUM") as l2p:
            accs = {}
            srcs = {0: h1tab0[0], 1: h1tab0[1], 2: h1tab1[0], 3: h1tab1[1]}

            def emit_chunk_group(st, c):
                tiles = cfg.tiles_of_st(st)
                if c == 0:
                    acc_st = accp.tile([P, len(tiles), F_HID], dt.float16,
                                       tag="accS", name=f"accS{st}")
                    accs[st] = acc_st
                colstart, G, nidx = st_meta2[st][c]
                idx_t = l2s.tile([P, G * 8], dt.int16, tag="idx")
                nc.sync.dma_start(idx_t[:], idx_in[:, colstart * 8:(colstart + G) * 8])
                gt = gpool.tile([P, G, F_HID], dt.float16, tag="gath")
                thin_gather(gt[:], srcs[c][:, 0:F_HID], idx_t[:], nidx)
                for ti, t in enumerate(tiles):
                    chs = tile_chunks2[(c, t)]
                    acc = l2p.tile([P, F_HID], dt.float32, tag="accq", bufs=4)
                    for ci, (col, kind, j, oidx) in enumerate(chs):
                        lhs = Msts[:, j, :] if kind == 's' else \
                            build_M(dv2s[:, oidx:oidx + 1])[:]
                        nc.tensor.matmul(
                            acc[:], lhsT=lhs, rhs=gt[:, col - colstart, :],
                            start=(ci == 0), stop=(ci == len(chs) - 1))
                    sl = accs[st][:, ti, :]
                    if c == 0:
                        # on DVE, not Act: keeps the Act queue free of phase-A
                        # work so a mis-scheduled expand cannot stall phase A
                        nc.vector.tensor_copy(sl, acc[:])
                    else:
                        nc.vector.tensor_add(sl, acc[:], sl)

            def emit_final(st):
                tiles = cfg.tiles_of_st(st)
                for ti, t in enumerate(tiles):
                    sl = accs[st][:, ti, :]
                    slf = evac.tile([P, F_HID], dt.float16, tag="slf")
                    nc.vector.tensor_scalar(out=slf[:], in0=sl, scalar1=snm[:, t:t + 1],
                                            scalar2=None, op0=mybir.AluOpType.mult)
                    aT = l2p.tile([F_HID, P], dt.float16, tag="aT", bufs=1)
                    nc.tensor.transpose(aT[:], slf[:], identP[:])
                    a2s = evac.tile([F_HID, P], dt.float16, tag="a2s")
                    nc.scalar.activation(a2s[:], aT[:], mybir.ActivationFunctionType.Copy)
                    ph2 = l2p.tile([F_HID, P], dt.float32, tag="ph2", bufs=2)
                    nc.tensor.matmul(ph2[:], lhsT=W2s16[:], rhs=a2s[:], start=True, stop=True)
                    h2f = evac.tile([F_HID, P], dt.float16, tag="h2f")
                    nc.scalar.activation(h2f[:], ph2[:], mybir.ActivationFunctionType.Relu,
                                         bias=b2s[:, 0:1])
                    po = l2p.tile([F_OUT, P], dt.float32, tag="po", bufs=1)
                    nc.tensor.matmul(po[:], lhsT=Wfs16[:], rhs=h2f[:], start=True, stop=True)
                    osb = evac.tile([F_OUT, P], dt.float32, tag="osb")
                    nc.scalar.activation(osb[:], po[:], mybir.ActivationFunctionType.Identity,
                                         bias=bfs[:, 0:1])
                    rows = cfg.rows_of_tile(t)
                    nc.sync.dma_start(out_ext[:, t * P:t * P + rows], osb[:, :rows])

            emit_expand(0)
            for st in range(cfg.n_st):
                emit_chunk_group(st, 0)
                emit_chunk_group(st, 1)
            # phase B depends on the second AllGather half; hold it back in the
            # scheduling sim so it cannot be interleaved ahead of phase A in
            # any engine queue (scheduler-only hint, no runtime wait)
            with tc.tile_wait_until(0.45):
                emit_expand(1)
                for st in range(cfg.n_st):
                    emit_chunk_group(st, 2)
                    emit_chunk_group(st, 3)
                    emit_final(st)

    nc.finalize()
    return nc


def make_in_maps(cfg, dev, wb):
    maps = []
    for cpu in range(cfg.n_cores):
        d = dev[cpu]
        maps.append({
            "xs": d["xs"], "dv1": d["dv1"], "idx": d["idx"], "dv2": d["dv2"],
            "s_nm": d["s_nm"],
            **{k: wb[k] for k in ("W1", "W2", "Wf", "B1", "b2", "bf", "Mst")},
        })
    return maps


def kernel(x, edge_index, W1, b1, W2, b2, Wf, bf, _trace=False, _tmpdir=None):
    from concourse.bass_utils import run_bass_kernel_spmd

    cfg = CFG
    dev, wb, sched = preprocess(cfg, x, edge_index, W1, b1, W2, b2, Wf, bf)
    nc = build(cfg, sched)
    in_maps = make_in_maps(cfg, dev, wb)
    res = run_bass_kernel_spmd(nc, in_maps, core_ids=list(range(cfg.n_cores)),
                               trace=_trace, tmpdir=_tmpdir)
    out = np.concatenate([res.results[c]["out_fm"].T for c in range(cfg.n_cores)], axis=0)
    kernel._last_results = res
    return out.astype(np.float32)


# revision 19
# speedup vs baseline: 1.9780x; 1.5136x over previous
"""Trainium2 Bass kernel for a 2-layer GCN (GCNConv -> relu -> GCNConv -> relu -> Linear).

Math: with s = deg^-1/2 (deg over dst incl. self-loops):
  h1 = relu( s_d * (A_ind @ (s_s * x)) @ W1 + b1 )   (aggregate 4-wide first)
  h2 = relu( W2 @ (s_d * (A_ind @ h1')) + b2 ),  h1' = h1 * s  (table prescale)
  out = h2 @ Wf + bf
A_ind is the pure 0/1 edge indicator: s_src is folded into the streamed
features (host), s_dst applied per dst-tile after aggregation.

Device strategy (8 cores, nodes sharded by dst, SPMD one program):
  - STRUCTURAL SLOTS: each dst gets a fixed quota of edge slots (L1: 32 per
    tile; L2: 8 per (dst, src_chunk)); a 128-slot chunk covers 16 consecutive
    dst x 8 slots, aggregated with a CONSTANT block-diagonal selection matrix
    Mst[:, j, :] (one per 16-dst slice, host-uploaded) -- no per-chunk M build.
    Overflow edges beyond the quota go through classic built-M chunks
    (DVE is_equal vs iota), but those are now rare (~15% of chunks).
  - slot padding: L1 pad slots stream zero features; L2 pad slots gather a
    reserved zero row (row 0 of each 25001-row table chunk, idx 0; real rows
    at idx 1+trow%25000).
  - AllGather moves only the 64 real fp16 feature columns (strided out AP into
    the 256B-row padded gather table) in 2 halves emitted inside the L1 loop.
  - layer-2 source rows fetched per-slot via raw InstDMAGatherAnt (elem_size
    64, 256B stride -> 128B payload/descriptor).
"""
import numpy as np
from contextlib import ExitStack
from dataclasses import dataclass


@dataclass(frozen=True)
class Cfg:
    n_nodes: int = 100000
    n_cores: int = 8
    f_in: int = 4
    f_hid: int = 64
    f_out: int = 2
    src_chunks: int = 4
    st_tiles: int = 4
    q1: int = 32   # L1 structural slots per dst (4 chunks x 8)
    q2: int = 8    # L2 structural slots per (dst, src_chunk) (1 chunk x 8)

    @property
    def shard(self):
        return self.n_nodes // self.n_cores

    @property
    def n_tiles(self):
        return (self.shard + 127) // 128

    @property
    def last_rows(self):
        return self.shard - (self.n_tiles - 1) * 128

    @property
    def n_st(self):
        return (self.n_tiles + self.st_tiles - 1) // self.st_tiles

    def tiles_of_st(self, st):
        return list(range(st * self.st_tiles, min((st + 1) * self.st_tiles, self.n_tiles)))

    def rows_of_tile(self, t):
        return self.last_rows if t == self.n_tiles - 1 else 128

    def slices_of_tile(self, t):
        return (self.rows_of_tile(t) + 15) // 16


CFG = Cfg()
P = 128
FP8_TABLE = True          # fp8 gather table: halves collective + gather bytes
FP = 256 if FP8_TABLE else 128   # table row elems padded to 256B stride
S = 8      # slots per dst per structural chunk
HS = CFG.shard // 2          # rows per AllGather half per core
CJ = HS * CFG.n_cores // 2   # 25000 rows per table idx chunk


def _ranks(sorted_keys):
    """rank of each element within its run of equal (sorted) keys."""
    n = len(sorted_keys)
    if n == 0:
        return np.zeros(0, dtype=np.int64)
    first = np.empty(n, dtype=bool)
    first[0] = True
    np.not_equal(sorted_keys[1:], sorted_keys[:-1], out=first[1:])
    gstart = np.flatnonzero(first)
    gid = np.cumsum(first) - 1
    return np.arange(n) - gstart[gid]


def preprocess(cfg, x, edge_index, W1, b1, W2, b2, Wf, bf):
    """Host-side sharding: slot assignment, overflow grouping, streams."""
    F_IN = cfg.f_in
    src0 = np.asarray(edge_index[0], dtype=np.int64)
    dst0 = np.asarray(edge_index[1], dtype=np.int64)
    deg = (np.bincount(dst0, minlength=cfg.n_nodes) + 1).astype(np.float64)
    s = (1.0 / np.sqrt(deg)).astype(np.float32)

    # self loops appended as ordinary edges
    loop = np.arange(cfg.n_nodes, dtype=np.int64)
    src = np.concatenate([src0, loop])
    dst = np.concatenate([dst0, loop])
    x = np.asarray(x, dtype=np.float32)
    xsrc_all = (x * s[:, None]).astype(np.float16)  # prescaled by s_src

    core_id = dst // cfg.shard

    # L2 table mapping for each global src node
    lr_all = src % cfg.shard
    score_all = src // cfg.shard
    h_all = lr_all // HS
    trow_all = score_all * HS + (lr_all - h_all * HS)
    c_all = h_all * 2 + trow_all // CJ          # src chunk 0..3
    idx_all = 1 + (trow_all % CJ)               # 1-based; 0 = zero row

    n_tiles, n_st = cfg.n_tiles, cfg.n_st

    # ---- pass 1: per-core sorted edge views + overflow counts ----
    cores = []
    ovf1 = np.zeros((cfg.n_cores, n_tiles), dtype=np.int64)
    ovf2 = np.zeros((cfg.n_cores, cfg.src_chunks * n_tiles), dtype=np.int64)
    for cpu in range(cfg.n_cores):
        m = core_id == cpu
        sc, dc = src[m], dst[m]
        dl = dc - cpu * cfg.shard
        tl = dl // P
        d128 = dl % P
        cch = c_all[m]
        idxv = idx_all[m]

        # L1: sort by (t, d128)
        o1 = np.lexsort((d128, tl))
        t1, dd1, s1v = tl[o1], d128[o1], sc[o1]
        r1 = _ranks(t1 * P + dd1)
        m1o = r1 >= cfg.q1
        ovf1[cpu] = np.bincount(t1[m1o], minlength=n_tiles)

        # L2: sort by (c, t, d128)
        o2 = np.lexsort((d128, tl, cch))
        c2, t2, dd2, i2 = cch[o2], tl[o2], d128[o2], idxv[o2]
        r2 = _ranks((c2 * n_tiles + t2) * P + dd2)
        m2o = r2 >= cfg.q2
        ovf2[cpu] = np.bincount((c2 * n_tiles + t2)[m2o],
                                minlength=cfg.src_chunks * n_tiles)
        cores.append((o1, t1, dd1, s1v, r1, o2, c2, t2, dd2, i2, r2, sc, dc))

    C1 = np.maximum(1, -(-ovf1.max(axis=0) // P))            # [n_tiles]
    C2 = np.maximum(1, -(-ovf2.max(axis=0) // P)).reshape(cfg.src_chunks, n_tiles)

    # ---- shared column layouts ----
    # L1: per st: for t: nS1(t) structural cols then C1(t) ovf cols
    nS1 = [4 * cfg.slices_of_tile(t) for t in range(n_tiles)]
    col1_struct = np.zeros(n_tiles, dtype=np.int64)   # base of structural cols
    col1_ovf = np.zeros(n_tiles, dtype=np.int64)
    tile_chunks1 = [[] for _ in range(n_tiles)]       # (col, kind, j, ovfidx)
    l1_st_cols = []
    ovfidx1 = np.zeros(n_tiles, dtype=np.int64)
    nc1 = 0
    nov1 = 0
    for st in range(n_st):
        st_start = nc1
        for t in cfg.tiles_of_st(st):
            col1_struct[t] = nc1
            for j in range(cfg.slices_of_tile(t)):
                for k in range(4):
                    tile_chunks1[t].append((nc1, 's', j, -1))
                    nc1 += 1
            col1_ovf[t] = nc1
            ovfidx1[t] = nov1
            for q in range(int(C1[t])):
                tile_chunks1[t].append((nc1, 'o', -1, nov1))
                nc1 += 1
                nov1 += 1
        l1_st_cols.append((st_start, nc1))
    NC1, NOV1 = nc1, nov1

    # L2: per (st, c): for t: q2//S structural cols + C2(c,t) ovf cols
    col2_struct = np.zeros((cfg.src_chunks, n_tiles), dtype=np.int64)
    col2_ovf = np.zeros((cfg.src_chunks, n_tiles), dtype=np.int64)
    ovfidx2 = np.zeros((cfg.src_chunks, n_tiles), dtype=np.int64)
    tile_chunks2 = {}                                 # (c,t) -> list
    st_meta2 = [[None] * cfg.src_chunks for _ in range(n_st)]
    nc2 = 0
    nov2 = 0
    for st in range(n_st):
        for c in range(cfg.src_chunks):
            colstart = nc2
            for t in cfg.tiles_of_st(st):
                lst = []
                col2_struct[c, t] = nc2
                for j in range(cfg.slices_of_tile(t)):
                    lst.append((nc2, 's', j, -1))
                    nc2 += 1
                col2_ovf[c, t] = nc2
                ovfidx2[c, t] = nov2
                for q in range(int(C2[c, t])):
                    lst.append((nc2, 'o', -1, nov2))
                    nc2 += 1
                    nov2 += 1
                tile_chunks2[(c, t)] = lst
            G = nc2 - colstart
            st_meta2[st][c] = (colstart, G, G * P)
    NC2, NOV2 = nc2, nov2

    # ---- pass 2: per-core device arrays ----
    dev = []
    for cpu in range(cfg.n_cores):
        (o1, t1, dd1, s1v, r1, o2, c2, t2, dd2, i2, r2, sc, dc) = cores[cpu]

        # L1 stream positions
        j1 = dd1 // 16
        p16_1 = dd1 % 16
        ms = r1 < cfg.q1
        col_s = col1_struct[t1[ms]] + j1[ms] * 4 + r1[ms] // S
        row_s = p16_1[ms] * S + r1[ms] % S
        pos_s = col_s * P + row_s
        mo = ~ms
        to = t1[mo]
        orank = _ranks(to)  # overflow edges sorted by t already
        col_o = col1_ovf[to] + orank // P
        row_o = orank % P
        pos_o = col_o * P + row_o

        xs1 = np.zeros((NC1 * P, F_IN), dtype=np.float16)
        xs1[pos_s] = xsrc_all[s1v[ms]]
        xs1[pos_o] = xsrc_all[s1v[mo]]
        dv1 = np.full(NOV1 * P, -1.0, dtype=np.float32)
        dv1[(ovfidx1[to] + orank // P) * P + row_o] = dd1[mo].astype(np.float32)

        xs = np.ascontiguousarray(xs1.reshape(NC1, P, F_IN).transpose(1, 0, 2))
        dv1w = np.ascontiguousarray(dv1.reshape(NOV1, P).T)

        # L2 stream positions
        j2 = dd2 // 16
        p16_2 = dd2 % 16
        ms2 = r2 < cfg.q2
        colb = col2_struct[c2[ms2], t2[ms2]] + j2[ms2]
        rowb = p16_2[ms2] * S + r2[ms2]
        pos2s = colb * P + rowb
        mo2 = ~ms2
        key_o2 = c2[mo2] * n_tiles + t2[mo2]
        orank2 = _ranks(key_o2)
        col_o2 = col2_ovf[c2[mo2], t2[mo2]] + orank2 // P
        row_o2 = orank2 % P
        pos2o = col_o2 * P + row_o2

        idx2 = np.zeros(NC2 * P, dtype=np.int16)
        idx2[pos2s] = i2[ms2].astype(np.int16)
        idx2[pos2o] = i2[mo2].astype(np.int16)
        dv2 = np.full(NOV2 * P, -1.0, dtype=np.float32)
        dv2[(ovfidx2[c2[mo2], t2[mo2]] + orank2 // P) * P + row_o2] = \
            dd2[mo2].astype(np.float32)

        idx_w = np.tile(idx2.reshape(NC2 * 8, 16).T, (8, 1))
        dv2w = np.ascontiguousarray(dv2.reshape(NOV2, P).T)

        s_core = np.zeros(n_tiles * P, dtype=np.float32)
        s_core[:cfg.shard] = s[cpu * cfg.shard:(cpu + 1) * cfg.shard]
        s_nm = s_core.reshape(n_tiles, P).T.copy()

        dev.append(dict(xs=xs, dv1=dv1w, idx=np.ascontiguousarray(idx_w),
                        dv2=dv2w, s_nm=s_nm))

    # structural selection matrices: Mst[r, j, c] = 1 if c == 16j + r//8
    Mst = np.zeros((P, 8, P), dtype=np.float16)
    r = np.arange(P)
    for j in range(8):
        Mst[r, j, 16 * j + r // S] = 1.0

    wb = dict(
        W1=np.asarray(W1, np.float32), W2=np.asarray(W2, np.float32),
        Wf=np.asarray(Wf, np.float32),
        B1=np.broadcast_to(np.asarray(b1, np.float32).reshape(1, cfg.f_hid),
                           (P, cfg.f_hid)).copy(),
        b2=np.asarray(b2, np.float32).reshape(cfg.f_hid, 1),
        bf=np.asarray(bf, np.float32).reshape(cfg.f_out, 1),
        Mst=Mst,
    )
    sched = dict(NC1=NC1, NOV1=NOV1, NC2=NC2, NOV2=NOV2,
                 l1_st_cols=l1_st_cols, tile_chunks1=tile_chunks1,
                 st_meta2=st_meta2, tile_chunks2=tile_chunks2)
    return dev, wb, sched


def build(cfg, sched):
    import concourse.bass as bass
    import concourse.mybir as mybir
    import concourse.tile as tile
    from concourse import bacc

    dt = mybir.dt
    F_IN, F_HID, F_OUT = cfg.f_in, cfg.f_hid, cfg.f_out
    NC1, NOV1 = sched["NC1"], sched["NOV1"]
    NC2, NOV2 = sched["NC2"], sched["NOV2"]
    l1_st_cols = sched["l1_st_cols"]
    tile_chunks1 = sched["tile_chunks1"]
    st_meta2 = sched["st_meta2"]
    tile_chunks2 = sched["tile_chunks2"]

    TDT = dt.float8e4 if FP8_TABLE else dt.float16   # table/gather dtype

    nc = bacc.Bacc("TRN2", target_bir_lowering=False, num_devices=cfg.n_cores)
    xs_in = nc.declare_dram_parameter("xs", [P, NC1, F_IN], dt.float16, isOutput=False)
    dv1_in = nc.declare_dram_parameter("dv1", [P, NOV1], dt.float32, isOutput=False)
    idx_in = nc.declare_dram_parameter("idx", [P, NC2 * 8], dt.int16, isOutput=False)
    dv2_in = nc.declare_dram_parameter("dv2", [P, NOV2], dt.float32, isOutput=False)
    snm_in = nc.declare_dram_parameter("s_nm", [P, cfg.n_tiles], dt.float32, isOutput=False)
    Mst_in = nc.declare_dram_parameter("Mst", [P, 8, P], dt.float16, isOutput=False)
    W1_in = nc.declare_dram_parameter("W1", [F_IN, F_HID], dt.float32, isOutput=False)
    W2_in = nc.declare_dram_parameter("W2", [F_HID, F_HID], dt.float32, isOutput=False)
    Wf_in = nc.declare_dram_parameter("Wf", [F_HID, F_OUT], dt.float32, isOutput=False)
    B1_in = nc.declare_dram_parameter("B1", [P, F_HID], dt.float32, isOutput=False)
    b2_in = nc.declare_dram_parameter("b2", [F_HID, 1], dt.float32, isOutput=False)
    bf_in = nc.declare_dram_parameter("bf", [F_OUT, 1], dt.float32, isOutput=False)
    out_ext = nc.declare_dram_parameter("out_fm", [F_OUT, cfg.shard], dt.float32, isOutput=True)

    def thin_gather(out_ap, in_ap, idxs_ap, num_idxs):
        """dma_gather fetching the first 128B of each 256B-strided table row."""
        eng = nc.gpsimd
        _in_ap = eng.lower_ap_dma(in_ap, for_custom_bir_dma=True)
        _idxs_ap = eng.lower_ap(idxs_ap)
        _out_ap = eng.lower_ap(out_ap)
        return eng.add_instruction(
            mybir.InstDMAGatherAnt(
                name=eng.bass.get_next_instruction_name(),
                ins=[*_in_ap, _idxs_ap, eng.lower_val_access(eng.to_reg(num_idxs))],
                outs=[_out_ap],
                transpose=False,
                num_idxs=num_idxs,
                elem_size=F_HID,
                stride_bytes_256=1,
                gen_mode=0,
                single_packet=False,
                queue_num=0,
                sbuf_tokens_per_rank=0,
                sbuf_free_dim_per_rank=0,
                sbuf_free_dim_pad_per_rank=0,
                sbuf_byte_offset=0,
            )
        )

    with tile.TileContext(nc, num_cores=cfg.n_cores) as tc, ExitStack() as ctx:
        dram = ctx.enter_context(tc.tile_pool(name="dram", bufs=1, space="DRAM"))
        const = ctx.enter_context(tc.tile_pool(name="const", bufs=1))
        mpool = ctx.enter_context(tc.tile_pool(name="mpool", bufs=12))
        evac = ctx.enter_context(tc.tile_pool(name="evac", bufs=6))

        h1loc = dram.tile([cfg.shard, F_HID], TDT)
        # gather tables: 2 halves x [2 chunks, 1 zero row + 25000 rows, FP]
        h1tab0 = dram.tile([2, CJ + 1, FP], TDT, name="h1tab0")
        h1tab1 = dram.tile([2, CJ + 1, FP], TDT, name="h1tab1")
        # compact AllGather landing buffers (collective outs must be contiguous)
        h1cmp0 = dram.tile([2, CJ, F_HID], TDT, name="h1cmp0")
        h1cmp1 = dram.tile([2, CJ, F_HID], TDT, name="h1cmp1")

        iota_i = const.tile([P, P], dt.int16)
        nc.gpsimd.iota(iota_i[:], pattern=[[1, P]], base=0, channel_multiplier=0)
        iota16 = const.tile([P, P], dt.float16)
        nc.vector.tensor_copy(iota16[:], iota_i[:])
        iotapP = const.tile([P, 1], dt.int16)
        nc.gpsimd.iota(iotapP[:], pattern=[[0, 1]], base=0, channel_multiplier=1)
        iotapPf = const.tile([P, 1], dt.float32)
        nc.vector.tensor_copy(iotapPf[:], iotapP[:])
        identP = const.tile([P, P], dt.float16)
        nc.vector.tensor_scalar(out=identP[:], in0=iota16[:], scalar1=iotapPf[:, 0:1],
                                scalar2=None, op0=mybir.AluOpType.is_equal)
        zrow = const.tile([2, FP], TDT)
        nc.vector.memset(zrow[:], 0.0)
        for half in range(2):
            nc.sync.dma_start(h1tab0[half, 0:1, :], zrow[half:half + 1, :])
            nc.sync.dma_start(h1tab1[half, 0:1, :], zrow[half:half + 1, :])

        W1s = const.tile([F_IN, F_HID], dt.float32)
        W2s = const.tile([F_HID, F_HID], dt.float32)
        Wfs = const.tile([F_HID, F_OUT], dt.float32)
        B1s = const.tile([P, F_HID], dt.float32)
        b2s = const.tile([F_HID, 1], dt.float32)
        bfs = const.tile([F_OUT, 1], dt.float32)
        snm = const.tile([P, cfg.n_tiles], dt.float32)
        Msts = const.tile([P, 8, P], dt.float16)
        Msts2 = const.tile([P, 8, P], TDT)
        dv1s = const.tile([P, NOV1], dt.float32)
        dv2s = const.tile([P, NOV2], dt.float32)
        nc.sync.dma_start(W1s[:], W1_in[:])
        nc.sync.dma_start(W2s[:], W2_in[:])
        nc.sync.dma_start(Wfs[:], Wf_in[:])
        nc.sync.dma_start(B1s[:], B1_in[:])
        nc.sync.dma_start(b2s[:], b2_in[:])
        nc.sync.dma_start(bfs[:], bf_in[:])
        nc.sync.dma_start(snm[:], snm_in[:])
        nc.sync.dma_start(Msts[:], Mst_in[:])
        nc.vector.tensor_copy(Msts2[:], Msts[:])
        nc.sync.dma_start(dv1s[:], dv1_in[:])
        nc.sync.dma_start(dv2s[:], dv2_in[:])

        W1s16 = const.tile([F_IN, F_HID], dt.float16)
        nc.scalar.activation(W1s16[:], W1s[:], mybir.ActivationFunctionType.Copy)
        W2s16 = const.tile([F_HID, F_HID], dt.float16)
        nc.scalar.activation(W2s16[:], W2s[:], mybir.ActivationFunctionType.Copy)
        Wfs16 = const.tile([F_HID, F_OUT], dt.float16)
        nc.scalar.activation(Wfs16[:], Wfs[:], mybir.ActivationFunctionType.Copy)

        def build_M(scol, mdt=dt.float16, tag="M"):
            """Indicator matrix [128e, 128d] = (iota == dstv) on DVE."""
            M16 = mpool.tile([P, P], mdt, tag=tag)
            nc.vector.tensor_scalar(
                out=M16[:], in0=iota16[:], scalar1=scol, scalar2=None,
                op0=mybir.AluOpType.is_equal)
            return M16

        # ---------------- layer 1 ----------------
        hs2 = cfg.shard // 2
        ag_after = {}
        for half, (a, b), tab, cmp_ in ((0, (0, hs2), h1tab0, h1cmp0),
                                        (1, (hs2, 2 * hs2), h1tab1, h1cmp1)):
            last_tile = (b - 1) // P
            st_of = last_tile // cfg.st_tiles
            ag_after[st_of] = ((a, b), tab, cmp_)

        expands = []

        def emit_ag(st):
            if st not in ag_after:
                return
            (a, b), tab, cmp_ = ag_after.pop(st)
            nc.gpsimd.collective_compute(
                "AllGather", mybir.AluOpType.bypass,
                replica_groups=[list(range(cfg.n_cores))],
                ins=[h1loc[a:b, :].opt()],
                outs=[cmp_[:].opt()],
            )
            # expand into the 256B-strided gather table is deferred: emitted on
            # the Act HWDGE queue at a point where its sem wait cannot block
            # queued work that must run before the collective completes.
            expands.append((tab, cmp_))

        def emit_expand(i):
            tab, cmp_ = expands[i]
            nc.scalar.dma_start(tab[:, 1:, 0:F_HID], cmp_[:])

        with tc.tile_pool(name="l1s", bufs=3) as l1s, \
             tc.tile_pool(name="l1p", bufs=2, space="PSUM") as l1p:
            for st in range(cfg.n_st):
                colstart, colend = l1_st_cols[st]
                n_stc = colend - colstart
                xs_st = l1s.tile([P, n_stc, F_IN], dt.float16, tag="xs")
                nc.sync.dma_start(xs_st[:], xs_in[:, colstart:colend, :])

                for t in cfg.tiles_of_st(st):
                    chunks = tile_chunks1[t]
                    acc1 = l1p.tile([P, F_IN], dt.float32, tag="acc1", bufs=3)
                    for ci, (col, kind, j, oidx) in enumerate(chunks):
                        lhs = Msts[:, j, :] if kind == 's' else \
                            build_M(dv1s[:, oidx:oidx + 1])[:]
                        nc.tensor.matmul(
                            acc1[:], lhsT=lhs, rhs=xs_st[:, col - colstart, :],
                            start=(ci == 0), stop=(ci == len(chunks) - 1))
                    # dense: scale by s_dst -> transpose -> W1 -> +b1, relu, *s
                    a1n = evac.tile([P, F_IN], dt.float16, tag="a1n")
                    nc.vector.tensor_scalar(out=a1n[:], in0=acc1[:],
                                            scalar1=snm[:, t:t + 1], scalar2=None,
                                            op0=mybir.AluOpType.mult)
                    a1T = l1p.tile([F_IN, P], dt.float16, tag="a1T", bufs=1)
                    nc.tensor.transpose(a1T[:], a1n[:], identP[:])
                    a1s = evac.tile([F_IN, P], dt.float16, tag="a1s")
                    nc.scalar.activation(a1s[:], a1T[:], mybir.ActivationFunctionType.Copy)
                    z1p = l1p.tile([P, F_HID], dt.float32, tag="z1p", bufs=2)
                    nc.tensor.matmul(z1p[:], lhsT=a1s[:], rhs=W1s16[:], start=True, stop=True)
                    t1 = evac.tile([P, F_HID], dt.float16, tag="t1")
                    nc.vector.tensor_tensor(out=t1[:], in0=z1p[:], in1=B1s[:],
                                            op=mybir.AluOpType.add)
                    h1k = evac.tile([P, F_HID], TDT, tag="h1k")
                    nc.vector.tensor_scalar(
                        out=h1k[:], in0=t1[:], scalar1=snm[:, t:t + 1],
                        scalar2=0.0, op0=mybir.AluOpType.mult, op1=mybir.AluOpType.max)
                    rows = cfg.rows_of_tile(t)
                    nc.sync.dma_start(h1loc[t * P:t * P + rows, :], h1k[:rows, :])
                emit_ag(st)

        # ---------------- layer 2 ----------------
        with tc.tile_pool(name="l2s", bufs=4) as l2s, \
             tc.tile_pool(name="gpool", bufs=5) as gpool, \
             tc.tile_pool(name="accp", bufs=cfg.n_st) as accp, \
             tc.tile_pool(name="l2p", bufs=1, space="PSUM") as l2p:
            accs = {}
            srcs = {0: h1tab0[0], 1: h1tab0[1], 2: h1tab1[0], 3: h1tab1[1]}

            def emit_chunk_group(st, c):
                tiles = cfg.tiles_of_st(st)
                if c == 0:
                    acc_st = accp.tile([P, len(tiles), F_HID], dt.float16,
                                       tag="accS", name=f"accS{st}")
                    accs[st] = acc_st
                colstart, G, nidx = st_meta2[st][c]
                idx_t = l2s.tile([P, G * 8], dt.int16, tag="idx")
                nc.sync.dma_start(idx_t[:], idx_in[:, colstart * 8:(colstart + G) * 8])
                gt = gpool.tile([P, G, F_HID], TDT, tag="gath")
                thin_gather(gt[:], srcs[c][:, 0:F_HID], idx_t[:], nidx)
                for ti, t in enumerate(tiles):
                    chs = tile_chunks2[(c, t)]
                    acc = l2p.tile([P, F_HID], dt.float32, tag="accq", bufs=4)
                    for ci, (col, kind, j, oidx) in enumerate(chs):
                        lhs = Msts2[:, j, :] if kind == 's' else \
                            build_M(dv2s[:, oidx:oidx + 1], TDT, tag="M2")[:]
                        nc.tensor.matmul(
                            acc[:], lhsT=lhs, rhs=gt[:, col - colstart, :],
                            start=(ci == 0), stop=(ci == len(chs) - 1))
                    sl = accs[st][:, ti, :]
                    if c == 0:
                        # on DVE, not Act: keeps the Act queue free of phase-A
                        # work so a mis-scheduled expand cannot stall phase A
                        nc.vector.tensor_copy(sl, acc[:])
                    else:
                        nc.vector.tensor_add(sl, acc[:], sl)

            def emit_final(st):
                tiles = cfg.tiles_of_st(st)
                for ti, t in enumerate(tiles):
                    sl = accs[st][:, ti, :]
                    slf = evac.tile([P, F_HID], dt.float16, tag="slf")
                    nc.vector.tensor_scalar(out=slf[:], in0=sl, scalar1=snm[:, t:t + 1],
                                            scalar2=None, op0=mybir.AluOpType.mult)
                    aT = l2p.tile([F_HID, P], dt.float16, tag="aT", bufs=1)
                    nc.tensor.transpose(aT[:], slf[:], identP[:])
                    a2s = evac.tile([F_HID, P], dt.float16, tag="a2s")
                    nc.scalar.activation(a2s[:], aT[:], mybir.ActivationFunctionType.Copy)
                    ph2 = l2p.tile([F_HID, P], dt.float32, tag="ph2", bufs=2)
                    nc.tensor.matmul(ph2[:], lhsT=W2s16[:], rhs=a2s[:], start=True, stop=True)
                    h2f = evac.tile([F_HID, P], dt.float16, tag="h2f")
                    nc.scalar.activation(h2f[:], ph2[:], mybir.ActivationFunctionType.Relu,
                                         bias=b2s[:, 0:1])
                    po = l2p.tile([F_OUT, P], dt.float32, tag="po", bufs=1)
                    nc.tensor.matmul(po[:], lhsT=Wfs16[:], rhs=h2f[:], start=True, stop=True)
                    osb = evac.tile([F_OUT, P], dt.float32, tag="osb")
                    nc.scalar.activation(osb[:], po[:], mybir.ActivationFunctionType.Identity,
                                         bias=bfs[:, 0:1])
                    rows = cfg.rows_of_tile(t)
                    nc.sync.dma_start(out_ext[:, t * P:t * P + rows], osb[:, :rows])

            emit_expand(0)
            for st in range(cfg.n_st):
                emit_chunk_group(st, 0)
                emit_chunk_group(st, 1)
            emit_expand(1)
            for st in range(cfg.n_st):
                emit_chunk_group(st, 2)
                emit_chunk_group(st, 3)
                emit_final(st)

    nc.finalize()
    return nc


def make_in_maps(cfg, dev, wb):
    maps = []
    for cpu in range(cfg.n_cores):
        d = dev[cpu]
        maps.append({
            "xs": d["xs"], "dv1": d["dv1"], "idx": d["idx"], "dv2": d["dv2"],
            "s_nm": d["s_nm"],
            **{k: wb[k] for k in ("W1", "W2", "Wf", "B1", "b2", "bf", "Mst")},
        })
    return maps


def kernel(x, edge_index, W1, b1, W2, b2, Wf, bf, _trace=False, _tmpdir=None):
    from concourse.bass_utils import run_bass_kernel_spmd

    cfg = CFG
    dev, wb, sched = preprocess(cfg, x, edge_index, W1, b1, W2, b2, Wf, bf)
    nc = build(cfg, sched)
    in_maps = make_in_maps(cfg, dev, wb)
    res = run_bass_kernel_spmd(nc, in_maps, core_ids=list(range(cfg.n_cores)),
                               trace=_trace, tmpdir=_tmpdir)
    out = np.concatenate([res.results[c]["out_fm"].T for c in range(cfg.n_cores)], axis=0)
    kernel._last_results = res
    return out.astype(np.float32)
